# revision 1
# baseline (speedup 1.0000x reference)
"""Trainium2 Bass kernel for a dense transformer block (pre-LN attention + GELU MLP).

Strategy: data-parallel over batch across 8 NeuronCores (2 batches/core, no
collectives).  Per core: token-major residual stream with feature-major
activations for matmuls (PE-transpose at the two LayerNorms), fp32r matmuls
(full PE rate), softmax without max-subtraction (scores are O(1) bounded by
construction), PV matmul with a ones-column on V to produce row-sums for free.
"""

import numpy as np

import concourse.bass as bass
import concourse.mybir as mybir
import concourse.tile as tile
from concourse import bacc, bass_utils
from concourse.masks import make_identity

# Problem shape (hardcoded per spec nn_Block_58652073394865)
B, S, D, H, F = 16, 577, 1024, 16, 4096
DH = D // H
NCORES = 8
BL = B // NCORES        # batches per core
P = 128
KK = D // P             # 8 chunks of the model dim
FK = F // P             # 32 chunks of the mlp dim
EPS = 1e-6

# fp32r matmuls require even free-dim counts, so pad tokens 577 -> 578 (one
# zeroed pad token) and use even, overlapping moving-token chunks.
SP = 578
TT = [(0, 128), (128, 128), (256, 128), (384, 128), (512, 66)]   # token tiles (incl pad)
QC = [(0, 290), (288, 290)]                                      # moving-token chunks (even, >=256)
DC = [(0, 512), (512, 512)]                                      # model-dim 512 chunks
VS = 66                                                          # per-head stride in v (64 v + 1 ones + 1 pad)

F32 = mybir.dt.float32
F32R = mybir.dt.float32r
AF = mybir.ActivationFunctionType
OP = mybir.AluOpType

WEIGHT_NAMES = [
    "ln1_g", "ln1_b", "wq", "bq", "wk", "bk", "wv", "bv", "wo", "bo",
    "ln2_g", "ln2_b", "w1", "b1", "w2", "b2",
]

_NC_CACHE = None
# CoreSim doesn't implement the Gelu LUT; tests may swap this for AF.Tanh
_GELU = AF.Gelu


def _build():
    nc = bacc.Bacc("TRN2", target_bir_lowering=False, debug=False,
                   num_devices=NCORES)

    x_d = nc.dram_tensor("x", [BL, S, D], F32, kind="ExternalInput").ap()
    y_d = nc.dram_tensor("y", [BL, S, D], F32, kind="ExternalOutput").ap()
    # weights consumed by matmuls -> declare fp32r (same bits as fp32)
    wq_d = nc.dram_tensor("wq", [D, D], F32R, kind="ExternalInput").ap()
    wk_d = nc.dram_tensor("wk", [D, D], F32R, kind="ExternalInput").ap()
    wv_d = nc.dram_tensor("wv", [D, D], F32R, kind="ExternalInput").ap()
    wo_d = nc.dram_tensor("wo", [D, D], F32R, kind="ExternalInput").ap()
    w1_d = nc.dram_tensor("w1", [D, F], F32R, kind="ExternalInput").ap()
    w2_d = nc.dram_tensor("w2", [F, D], F32R, kind="ExternalInput").ap()
    bv_d = nc.dram_tensor("bv", [D], F32R, kind="ExternalInput").ap()   # folded via K=1 matmul
    bo_d = nc.dram_tensor("bo", [D], F32R, kind="ExternalInput").ap()   # folded via K=1 matmul
    bq_d = nc.dram_tensor("bq", [D], F32, kind="ExternalInput").ap()
    bk_d = nc.dram_tensor("bk", [D], F32, kind="ExternalInput").ap()
    b1_d = nc.dram_tensor("b1", [F], F32, kind="ExternalInput").ap()
    b2_d = nc.dram_tensor("b2", [D], F32, kind="ExternalInput").ap()
    g1_d = nc.dram_tensor("ln1_g", [D], F32, kind="ExternalInput").ap()
    gb1_d = nc.dram_tensor("ln1_b", [D], F32, kind="ExternalInput").ap()
    g2_d = nc.dram_tensor("ln2_g", [D], F32, kind="ExternalInput").ap()
    gb2_d = nc.dram_tensor("ln2_b", [D], F32, kind="ExternalInput").ap()

    wq_r = wq_d.rearrange("(ko p) d -> p ko d", p=P)
    wk_r = wk_d.rearrange("(ko p) d -> p ko d", p=P)
    wv_r = wv_d.rearrange("(ko p) d -> p ko d", p=P)
    wo_r = wo_d.rearrange("(ko p) d -> p ko d", p=P)
    w1_r = w1_d.rearrange("(ko p) d -> p ko d", p=P)
    w2_r = w2_d.rearrange("(ko p) d -> p ko d", p=P)

    with tile.TileContext(nc) as tc:
        with tc.tile_pool(name="const", bufs=1) as cpool, \
             tc.tile_pool(name="resid", bufs=2) as rpool, \
             tc.tile_pool(name="fmbuf", bufs=1) as fmpool, \
             tc.tile_pool(name="ostg", bufs=4) as opool, \
             tc.tile_pool(name="lnp", bufs=2) as lnpool, \
             tc.tile_pool(name="psA", bufs=4, space="PSUM") as psA:

            # ---- constants / small params ----
            # tiles pad to 4KB/partition: pack the small params into few tiles
            cA = cpool.tile([P, 7 * KK + FK], F32, tag="cA")
            bq_sb = cA[:, 0:KK]
            bk_sb = cA[:, KK:2 * KK]
            b2_sb = cA[:, 2 * KK:3 * KK]
            g1_sb = cA[:, 3 * KK:4 * KK]
            gb1_sb = cA[:, 4 * KK:5 * KK]
            g2_sb = cA[:, 5 * KK:6 * KK]
            gb2_sb = cA[:, 6 * KK:7 * KK]
            b1_sb = cA[:, 7 * KK:7 * KK + FK]
            nc.sync.dma_start(bq_sb, bq_d.rearrange("(m p) -> p m", p=P))
            nc.sync.dma_start(bk_sb, bk_d.rearrange("(m p) -> p m", p=P))
            nc.sync.dma_start(b2_sb, b2_d.rearrange("(m p) -> p m", p=P))
            nc.sync.dma_start(g1_sb, g1_d.rearrange("(c p) -> p c", p=P))
            nc.sync.dma_start(gb1_sb, gb1_d.rearrange("(c p) -> p c", p=P))
            nc.sync.dma_start(g2_sb, g2_d.rearrange("(c p) -> p c", p=P))
            nc.sync.dma_start(gb2_sb, gb2_d.rearrange("(c p) -> p c", p=P))
            nc.sync.dma_start(b1_sb, b1_d.rearrange("(m p) -> p m", p=P))

            cB = cpool.tile([P, P + 2], F32, tag="cB")
            ident = cB[:, 0:P]
            epsap = cB[:, P:P + 1]
            onec_f = cB[:, P + 1:P + 2]
            make_identity(nc, ident)
            nc.vector.memset(epsap, EPS)
            nc.vector.memset(onec_f, 1.0)

            ident_r = cpool.tile([P, P], F32R, tag="ident_r")
            nc.vector.tensor_copy(ident_r[:], ident)

            ones_f = cpool.tile([1, P], F32, tag="ones_f")
            nc.vector.memset(ones_f[:], 1.0)
            cD = cpool.tile([1, P + 2 * D], F32R, tag="cD")
            ones_r = cD[:, 0:P]
            t_bo = cD[:, P:P + D]
            t_bv = cD[:, P + D:P + 2 * D]
            nc.vector.tensor_copy(ones_r, ones_f[:])
            nc.sync.dma_start(t_bo, bo_d[None, :])
            nc.sync.dma_start(t_bv, bv_d[None, :])

            # token-major layernorm -> feature-major normalized output
            def ln_new_stats(ln_pool):
                stats = ln_pool.tile([P, 20], F32, tag="stats")
                # last token tile covers only 66 partitions; keep the rest defined
                nc.vector.memset(stats[:, 0:5], 0.0)
                nc.vector.memset(stats[:, 5:10], 1.0)
                return stats

            def ln_tile_stats(ln_pool, stats, src, ti, pt):
                negmu = stats[:, 0:5]
                varD = stats[:, 5:10]
                nc.vector.tensor_reduce(
                    negmu[:pt, ti:ti + 1], src[:pt, ti],
                    mybir.AxisListType.X, OP.add)
                nc.vector.tensor_scalar_mul(
                    negmu[:pt, ti:ti + 1], negmu[:pt, ti:ti + 1], -1.0 / D)
                scr = ln_pool.tile([P, D], F32R, tag="xn_tm", bufs=3)
                nc.scalar.activation(
                    scr[:pt], src[:pt, ti], AF.Square,
                    bias=negmu[:pt, ti:ti + 1], accum_out=varD[:pt, ti:ti + 1])

            def ln_finalize(stats, lo, hi):
                # rsig for tile range [lo, hi)
                nc.scalar.activation(stats[:, 10 + lo:10 + hi],
                                     stats[:, 5 + lo:5 + hi], AF.Sqrt,
                                     scale=1.0 / D, bias=epsap[:])
                nc.vector.reciprocal(stats[:, 15 + lo:15 + hi],
                                     stats[:, 10 + lo:10 + hi])

            def ln_apply_tiles(ln_pool, stats, src, g_sb, gb_sb, dst_fm, tis):
                negmu = stats[:, 0:5]
                rsig = stats[:, 15:20]
                for ti in tis:
                    t0, pt = TT[ti]
                    xn = ln_pool.tile([P, D], F32R, tag="xn_tm", bufs=3)
                    nc.vector.tensor_scalar(
                        xn[:pt], src[:pt, ti],
                        negmu[:pt, ti:ti + 1], rsig[:pt, ti:ti + 1],
                        OP.add, OP.mult)
                    for kk in range(KK):
                        pst = psA.tile([P, 512], F32R, tag="pA")
                        nc.tensor.transpose(
                            pst[:, :pt], xn[:pt, kk * P:(kk + 1) * P],
                            ident_r[:pt, :pt])
                        nc.vector.scalar_tensor_tensor(
                            dst_fm[:, kk, t0:t0 + pt], pst[:, :pt],
                            g_sb[:, kk:kk + 1],
                            gb_sb[:, kk:kk + 1].to_broadcast((P, pt)),
                            OP.mult, OP.add)

            def layer_norm_fm(ln_pool, src, g_sb, gb_sb, dst_fm):
                stats = ln_new_stats(ln_pool)
                for ti, (t0, pt) in enumerate(TT):
                    ln_tile_stats(ln_pool, stats, src, ti, pt)
                # finalize tile 0 alone so its transposes start after one x-tile
                ln_finalize(stats, 0, 1)
                ln_apply_tiles(ln_pool, stats, src, g_sb, gb_sb, dst_fm, (0,))
                ln_finalize(stats, 1, 4)
                ln_apply_tiles(ln_pool, stats, src, g_sb, gb_sb, dst_fm, (1, 2, 3))
                ln_finalize(stats, 4, 5)
                ln_apply_tiles(ln_pool, stats, src, g_sb, gb_sb, dst_fm, (4,))

            for b in range(BL):
                xn_fm = fmpool.tile([P, KK, SP], F32R, tag="xn_fm")
                xb = rpool.tile([P, 5, D], F32, tag="resid")

                # ---- stage A: load x (token-major); zero the pad token row ----
                # (engine start-partition must be a multiple of 32: zero 64..127
                # first, then the DMA rewrites the real rows 0..64)
                nc.vector.memset(xb[64:, 4, :], 0.0)
                for ti, (t0, pt) in enumerate(TT):
                    rp = min(pt, S - t0)   # real (non-pad) tokens in this tile
                    nc.sync.dma_start(xb[:rp, ti], x_d[b, t0:t0 + rp, :])

                # ---- stage B: LN1 -> xn_fm ----
                layer_norm_fm(lnpool, xb, g1_sb, gb1_sb, xn_fm)

                with tc.tile_pool(name="attn", bufs=1) as apool, \
                     tc.tile_pool(name="wblk", bufs=2) as wpool:
                    q_fm = apool.tile([P, KK, SP], F32R, tag="q")
                    k_fm = apool.tile([P, KK, SP], F32R, tag="k")
                    v_sb = apool.tile([P, 5, H * VS], F32R, tag="v")
                    ctx_fm = apool.tile([P, KK, SP], F32R, tag="ctx")

                    # col 64 of each head's stride-66 group = 1 (rowsum trick),
                    # col 65 = 0 (fp32r even-M pad).  The pad token's whole v
                    # row (tile 4, partition 65) must be zero: zero partitions
                    # 64.. first, later writes refill only the real rows.
                    v_hc = v_sb[:].rearrange("p t (h c) -> p t h c", c=VS)
                    # memset can't target fp32r; zero via a uint32 view
                    nc.vector.memset(v_hc[64:, 4:5].bitcast(mybir.dt.uint32), 0)
                    nc.vector.memset(v_hc[:, :, :, 65:66].bitcast(mybir.dt.uint32), 0)
                    nc.vector.tensor_copy(
                        v_hc[:, 0:4, :, 64:65],
                        onec_f[:, :, None, None].to_broadcast((P, 4, H, 1)))
                    nc.vector.tensor_copy(
                        v_hc[:65, 4:5, :, 64:65],
                        onec_f[:65, :, None, None].to_broadcast((65, 1, H, 1)))

                    # ---- stage C/D interleaved: projections + attention ----
                    # blk covers q/k m-tiles 4*blk..4*blk+3 and v heads
                    # 8*blk..8*blk+7 == attention heads 8*blk..8*blk+7, so each
                    # half's projections feed its attention while the NEXT
                    # half's projection matmuls fill the exp-bound PE idle.
                    def emit_qk(blk):
                        for w_r, bias_sb, dst in ((wq_r, bq_sb, q_fm), (wk_r, bk_sb, k_fm)):
                            wb = wpool.tile([P, KK, 512], F32R, tag="wblk")
                            nc.sync.dma_start(wb[:], w_r[:, :, blk * 512:(blk + 1) * 512])
                            for mi in range(4):
                                m = blk * 4 + mi
                                for (q0, qn) in QC:
                                    ps = psA.tile([P, 512], F32, tag="pA")
                                    for kk in range(KK):
                                        nc.tensor.matmul(
                                            ps[:, :qn],
                                            wb[:, kk, mi * P:(mi + 1) * P],
                                            xn_fm[:, kk, q0:q0 + qn],
                                            start=(kk == 0), stop=(kk == KK - 1))
                                    nc.scalar.activation(
                                        dst[:, m, q0:q0 + qn], ps[:, :qn],
                                        AF.Identity, bias=bias_sb[:, m:m + 1])

                    def emit_v(ci):
                        c0, cn = DC[ci]
                        wb = wpool.tile([P, KK, 512], F32R, tag="wblk")
                        nc.sync.dma_start(wb[:], wv_r[:, :, c0:c0 + cn])
                        for ti, (t0, pt) in enumerate(TT):
                            ps = psA.tile([P, 512], F32, tag="pA")
                            for kk in range(KK):
                                nc.tensor.matmul(
                                    ps[:pt], xn_fm[:, kk, t0:t0 + pt],
                                    wb[:, kk, :], start=(kk == 0), stop=False)
                            nc.tensor.matmul(
                                ps[:pt], ones_r[:, :pt], t_bv[:, c0:c0 + cn],
                                start=False, stop=True)
                            rp = min(pt, S - t0)
                            nc.vector.tensor_copy(
                                v_sb[:rp, ti].rearrange("p (h c) -> p h c", c=VS)[:, ci * 8:(ci + 1) * 8, 0:64],
                                ps[:rp, :cn].rearrange("p (h c) -> p h c", c=64))

                    def emit_attn(h):
                        hrow = (h % 2) * 64
                        kkh = h // 2
                        for qi, (q0, qn) in enumerate(QC):
                            es = apool.tile([P, 5, qn], F32R, tag=f"es{qi}")
                            # pair the 5 score tiles into 2-bank psum groups so
                            # each Exp covers 2 tiles (halves the per-op cost)
                            for pair in ((0, 1), (2, 3), (4,)):
                                pg = psA.tile([P, 2, 512], F32, tag="pS", bufs=2)
                                for j, kt in enumerate(pair):
                                    t0, ptk = TT[kt]
                                    nc.tensor.matmul(
                                        pg[:ptk, j, :qn],
                                        k_fm[hrow:hrow + 64, kkh, t0:t0 + ptk],
                                        q_fm[hrow:hrow + 64, kkh, q0:q0 + qn],
                                        start=True, stop=True)
                                npair = len(pair)
                                prow = TT[pair[0]][1]   # 128 for full pairs, 66 for (4,)
                                nc.scalar.activation(
                                    es[:prow, pair[0]:pair[0] + npair, :],
                                    pg[:prow, :npair, :qn],
                                    AF.Exp, scale=1.0 / np.sqrt(DH))
                            pc = psA.tile([VS, 512], F32, tag="pA")
                            for kt, (t0, ptk) in enumerate(TT):
                                nc.tensor.matmul(
                                    pc[:, :qn],
                                    v_sb[:ptk, kt, h * VS:(h + 1) * VS],
                                    es[:ptk, kt, :],
                                    start=(kt == 0), stop=(kt == 4))
                            rc = apool.tile([1, 290], F32, tag="rc", bufs=2)
                            nc.vector.reciprocal(rc[:, :qn], pc[64:65, :qn])
                            rb = apool.tile([64, 290], F32, tag="rb", bufs=2)
                            nc.gpsimd.partition_broadcast(rb[:, :qn], rc[:, :qn])
                            nc.vector.tensor_tensor(
                                ctx_fm[hrow:hrow + 64, kkh, q0:q0 + qn],
                                pc[0:64, :qn], rb[:, :qn], OP.mult)

                    emit_qk(0)
                    emit_v(0)
                    for h in range(8):
                        emit_attn(h)
                    emit_qk(1)
                    emit_v(1)
                    for h in range(8, H):
                        emit_attn(h)

                    # ---- stage E: output projection + residual -> x2,
                    # with LN2 folded in per-tile ----
                    x2 = rpool.tile([P, 5, D], F32, tag="resid")
                    xn2_fm = fmpool.tile([P, KK, SP], F32R, tag="xn_fm")
                    stats2 = ln_new_stats(lnpool)
                    for ci, (c0, cn) in enumerate(DC):
                        wb = wpool.tile([P, KK, 512], F32R, tag="wblk")
                        nc.sync.dma_start(wb[:], wo_r[:, :, c0:c0 + cn])
                        for ti, (t0, pt) in enumerate(TT):
                            ps = psA.tile([P, 512], F32, tag="pA")
                            for kk in range(KK):
                                nc.tensor.matmul(
                                    ps[:pt], ctx_fm[:, kk, t0:t0 + pt],
                                    wb[:, kk, :], start=(kk == 0), stop=False)
                            nc.tensor.matmul(
                                ps[:pt], ones_r[:, :pt], t_bo[:, c0:c0 + cn],
                                start=False, stop=True)
                            nc.vector.scalar_tensor_tensor(
                                x2[:pt, ti, c0:c0 + cn], ps[:pt], 0.0,
                                xb[:pt, ti, c0:c0 + cn], OP.add, OP.add)
                            if ci == len(DC) - 1:
                                # x2 tile complete: fold its LN2 stats in now
                                ln_tile_stats(lnpool, stats2, x2, ti, pt)



                # ---- stage F: LN2 apply ----
                ln_finalize(stats2, 0, 4)
                ln_apply_tiles(lnpool, stats2, x2, g2_sb, gb2_sb, xn2_fm, (0, 1, 2, 3))
                ln_finalize(stats2, 4, 5)
                ln_apply_tiles(lnpool, stats2, x2, g2_sb, gb2_sb, xn2_fm, (4,))

                # ---- stage G: MLP ----
                with tc.tile_pool(name="mlp", bufs=1) as mpool, \
                     tc.tile_pool(name="wmlp", bufs=2) as mwpool:
                    h1 = mpool.tile([P, FK, SP], F32R, tag="h1")
                    _psc = [0]

                    def mlp_psum():
                        # pS's 2x2 banks are idle during MLP: every 3rd group
                        # borrows one -> 6 accumulation groups in flight
                        _psc[0] += 1
                        if _psc[0] % 3 == 0:
                            t = psA.tile([P, 2, 512], F32, tag="pS", bufs=2,
                                         name="ps_alt")
                            return t[:, 0]
                        return psA.tile([P, 512], F32, tag="pA", name="ps_a")

                    for blk in range(8):
                        wb = mwpool.tile([P, KK, 512], F32R, tag="wmlp")
                        nc.sync.dma_start(wb[:], w1_r[:, :, blk * 512:(blk + 1) * 512])
                        for mi in range(4):
                            m = blk * 4 + mi
                            for (q0, qn) in QC:
                                ps = mlp_psum()
                                for kk in range(KK):
                                    nc.tensor.matmul(
                                        ps[:, :qn],
                                        wb[:, kk, mi * P:(mi + 1) * P],
                                        xn2_fm[:, kk, q0:q0 + qn],
                                        start=(kk == 0), stop=(kk == KK - 1))
                                nc.scalar.activation(
                                    h1[:, m, q0:q0 + qn], ps[:, :qn],
                                    _GELU, bias=b1_sb[:, m:m + 1])
                    mlp_fm = mpool.tile([P, KK, SP], F32R, tag="mlp_fm")
                    for m in range(KK):
                        wb = mwpool.tile([P, FK, P], F32R, tag="wmlp")
                        nc.sync.dma_start(wb[:], w2_r[:, :, m * P:(m + 1) * P])
                        for (q0, qn) in QC:
                            ps = mlp_psum()
                            for kk2 in range(FK):
                                nc.tensor.matmul(
                                    ps[:, :qn], wb[:, kk2],
                                    h1[:, kk2, q0:q0 + qn],
                                    start=(kk2 == 0), stop=(kk2 == FK - 1))
                            nc.vector.tensor_scalar_add(
                                mlp_fm[:, m, q0:q0 + qn], ps[:, :qn],
                                b2_sb[:, m:m + 1])
                        # this m's feature rows are complete: transpose back to
                        # token-major, add residual, store (interleaves with the
                        # next m's w2 matmuls)
                        for ti, (t0, pt) in enumerate(TT):
                            rp = min(pt, S - t0)   # skip the pad token on store
                            ps = psA.tile([P, 512], F32R, tag="pA")
                            nc.tensor.transpose(
                                ps[:pt, :P], mlp_fm[:, m, t0:t0 + pt], ident_r[:])
                            og = opool.tile([P, P], F32, tag="ostg", bufs=6)
                            nc.vector.scalar_tensor_tensor(
                                og[:pt], ps[:pt, :P], 0.0,
                                x2[:pt, ti, m * P:(m + 1) * P], OP.add, OP.add)
                            nc.sync.dma_start(
                                y_d[b, t0:t0 + rp, m * P:(m + 1) * P], og[:rp])

    nc.compile()
    return nc


def _get_nc():
    global _NC_CACHE
    if _NC_CACHE is None:
        _NC_CACHE = _build()
    return _NC_CACHE


def kernel(**inputs):
    nc = _get_nc()
    x = np.ascontiguousarray(np.asarray(inputs["x"], dtype=np.float32))
    shared = {
        n: np.ascontiguousarray(np.asarray(inputs[n], dtype=np.float32))
        for n in WEIGHT_NAMES
    }
    in_maps = []
    for i in range(NCORES):
        m = dict(shared)
        m["x"] = np.ascontiguousarray(x[i * BL:(i + 1) * BL])
        in_maps.append(m)
    res = bass_utils.run_bass_kernel_spmd(nc, in_maps, core_ids=list(range(NCORES)))
    y = np.concatenate([res.results[i]["y"] for i in range(NCORES)], axis=0)
    return y.astype(np.float32)



# revision 10
# speedup vs baseline: 1.0921x; 1.0921x over previous
"""Trainium2 Bass kernel for a dense transformer block (pre-LN attention + GELU MLP).

Strategy: data-parallel over batch across 8 NeuronCores (2 batches/core, no
collectives).  Mixed precision tuned to the TRN2 PE rates: attention matmuls in
fp8e4 with DoubleRow perf mode (2 K-tiles per instruction at 0.5 cycles/row),
MLP + transposes in bf16 (1.0), residual stream bf16, all accumulation fp32 in
PSUM.  Softmax uses exp(s/8 - 3) without max-subtraction (scores O(1) bounded)
and gets row sums for free from a ones-column in V via an es-stationary PV
matmul.  MLP of batch b-1 is emitted interleaved with attention of batch b so
the scalar-engine Exp stream hides under MLP matmuls.
"""

import numpy as np
import ml_dtypes

import concourse.bass as bass
import concourse.mybir as mybir
import concourse.tile as tile
from concourse import bacc, bass_utils
from concourse.masks import make_identity

# Problem shape (hardcoded per spec nn_Block_58652073394865)
B, S, D, H, F = 16, 577, 1024, 16, 4096
DH = D // H
NCORES = 8
BL = B // NCORES        # batches per core
P = 128
KK = D // P             # 8 chunks of the model dim
FK = F // P             # 32 chunks of the mlp dim
EPS = 1e-6

SP = 592                # token-dim padding: DoubleRow stationary APs need the
                        # slot-pair stride to be 16-byte aligned (592 = 37*16)
TT = [(0, 128), (128, 128), (256, 128), (384, 128), (512, 66)]   # token tiles
QC = [(0, 289), (289, 289)]                                      # moving halves
VS = 65                                                          # 64 v dims + ones col

WQ_SCALE = 32.0         # host-side weight scale into fp8 (w*32 ~ N(0,1))
V_SCALE = 16.0          # v stored as 16*v in fp8
CTX_SCALE = 16.0        # ctx stored as 16*ctx in fp8

F32 = mybir.dt.float32
BF16 = mybir.dt.bfloat16
FP8 = mybir.dt.float8e4
U8 = mybir.dt.uint8
DR = mybir.MatmulPerfMode.DoubleRow
AF = mybir.ActivationFunctionType
OP = mybir.AluOpType

E4NP = ml_dtypes.float8_e4m3
BFNP = ml_dtypes.bfloat16

ONE_FP8_BYTE = int(np.array(1.0, E4NP).view(np.uint8))  # e4m3 encoding of 1.0

_NC_CACHE = None
_GELU = AF.Gelu


def _build():
    nc = bacc.Bacc("TRN2", target_bir_lowering=False, debug=False,
                   num_devices=NCORES)

    x_d = nc.dram_tensor("x", [BL, S, D], BF16, kind="ExternalInput").ap()
    y_d = nc.dram_tensor("y", [BL, S, D], F32, kind="ExternalOutput").ap()
    wq_d = nc.dram_tensor("wq8", [D, D], FP8, kind="ExternalInput").ap()
    wk_d = nc.dram_tensor("wk8", [D, D], FP8, kind="ExternalInput").ap()
    wv_d = nc.dram_tensor("wv8", [D, D], FP8, kind="ExternalInput").ap()
    wo_d = nc.dram_tensor("wo8", [D, D], FP8, kind="ExternalInput").ap()
    w1_d = nc.dram_tensor("w1b", [D, F], BF16, kind="ExternalInput").ap()
    w2_d = nc.dram_tensor("w2b", [F, D], BF16, kind="ExternalInput").ap()
    bv_d = nc.dram_tensor("bv8", [D], FP8, kind="ExternalInput").ap()   # 32*bv
    bo_d = nc.dram_tensor("bo8", [D], FP8, kind="ExternalInput").ap()   # 512*bo
    bq_d = nc.dram_tensor("bq", [D], F32, kind="ExternalInput").ap()
    bk_d = nc.dram_tensor("bk", [D], F32, kind="ExternalInput").ap()
    b1_d = nc.dram_tensor("b1", [F], F32, kind="ExternalInput").ap()
    b2_d = nc.dram_tensor("b2", [D], F32, kind="ExternalInput").ap()
    g1_d = nc.dram_tensor("ln1_g", [D], F32, kind="ExternalInput").ap()
    gb1_d = nc.dram_tensor("ln1_b", [D], F32, kind="ExternalInput").ap()
    g2_d = nc.dram_tensor("ln2_g", [D], F32, kind="ExternalInput").ap()
    gb2_d = nc.dram_tensor("ln2_b", [D], F32, kind="ExternalInput").ap()

    wq_r = wq_d.rearrange("(ko p) d -> p ko d", p=P)
    wk_r = wk_d.rearrange("(ko p) d -> p ko d", p=P)
    wv_r = wv_d.rearrange("(ko p) d -> p ko d", p=P)
    wo_r = wo_d.rearrange("(ko p) d -> p ko d", p=P)
    w1_r = w1_d.rearrange("(ko p) d -> p ko d", p=P)
    w2_r = w2_d.rearrange("(ko p) d -> p ko d", p=P)

    with tile.TileContext(nc) as tc:
        with tc.tile_pool(name="const", bufs=1) as cpool, \
             tc.tile_pool(name="resid", bufs=2) as rpool, \
             tc.tile_pool(name="xnp", bufs=2) as xnpool, \
             tc.tile_pool(name="xn2p", bufs=2) as xn2pool, \
             tc.tile_pool(name="attn", bufs=1) as apool, \
             tc.tile_pool(name="esp", bufs=2) as espool, \
             tc.tile_pool(name="mlp", bufs=1) as mpool, \
             tc.tile_pool(name="wqk", bufs=2) as wpool, \
             tc.tile_pool(name="wm1", bufs=2) as m1pool, \
             tc.tile_pool(name="wm2", bufs=2) as m2pool, \
             tc.tile_pool(name="ostg", bufs=6) as opool, \
             tc.tile_pool(name="lnp", bufs=2) as lnpool, \
             tc.tile_pool(name="rcp", bufs=4) as rcpool, \
             tc.tile_pool(name="psA", bufs=4, space="PSUM") as psA, \
             tc.tile_pool(name="psS", bufs=2, space="PSUM") as psS:

            # ---- constants / small params ----
            cA = cpool.tile([P, 7 * KK + FK], F32, tag="cA")
            bq_sb = cA[:, 0:KK]
            bk_sb = cA[:, KK:2 * KK]
            b2_sb = cA[:, 2 * KK:3 * KK]
            g1_sb = cA[:, 3 * KK:4 * KK]
            gb1_sb = cA[:, 4 * KK:5 * KK]
            g2_sb = cA[:, 5 * KK:6 * KK]
            gb2_sb = cA[:, 6 * KK:7 * KK]
            b1_sb = cA[:, 7 * KK:7 * KK + FK]
            nc.sync.dma_start(bq_sb, bq_d.rearrange("(m p) -> p m", p=P))
            nc.sync.dma_start(bk_sb, bk_d.rearrange("(m p) -> p m", p=P))
            nc.sync.dma_start(b2_sb, b2_d.rearrange("(m p) -> p m", p=P))
            nc.sync.dma_start(g1_sb, g1_d.rearrange("(c p) -> p c", p=P))
            nc.sync.dma_start(gb1_sb, gb1_d.rearrange("(c p) -> p c", p=P))
            nc.sync.dma_start(g2_sb, g2_d.rearrange("(c p) -> p c", p=P))
            nc.sync.dma_start(gb2_sb, gb2_d.rearrange("(c p) -> p c", p=P))
            nc.sync.dma_start(b1_sb, b1_d.rearrange("(m p) -> p m", p=P))

            cB = cpool.tile([P, P + 2], F32, tag="cB")
            identf = cB[:, 0:P]
            epsap = cB[:, P:P + 1]
            nm3 = cB[:, P + 1:P + 2]
            make_identity(nc, identf)
            nc.vector.memset(epsap, EPS)
            nc.vector.memset(nm3, -3.0)

            identb = cpool.tile([P, P], BF16, tag="identb")
            nc.vector.tensor_copy(identb[:], identf)
            ident8 = cpool.tile([P, P], FP8, tag="ident8")
            nc.vector.tensor_copy(ident8[:], identf)

            # fp8 bias rows for the K=1 bias matmuls (token-major outputs)
            cD = cpool.tile([1, 2 * D + P], FP8, tag="cD")
            t_bv = cD[:, 0:D]
            t_bo = cD[:, D:2 * D]
            ones8 = cD[:, 2 * D:2 * D + P]
            nc.sync.dma_start(t_bv, bv_d[None, :])
            nc.sync.dma_start(t_bo, bo_d[None, :])
            nc.vector.memset(ones8.bitcast(U8), ONE_FP8_BYTE)

            # ---- layernorm helpers (token-major stats, feature-major out) ----
            def ln_new_stats():
                stats = lnpool.tile([P, 20], F32, tag="stats")
                nc.vector.memset(stats[:, 0:5], 0.0)
                nc.vector.memset(stats[:, 5:10], 1.0)
                return stats

            def ln_tile_stats(stats, src, ti, pt):
                negmu = stats[:, 0:5]
                varD = stats[:, 5:10]
                nc.vector.tensor_reduce(
                    negmu[:pt, ti:ti + 1], src[:pt, ti],
                    mybir.AxisListType.X, OP.add)
                nc.vector.tensor_scalar_mul(
                    negmu[:pt, ti:ti + 1], negmu[:pt, ti:ti + 1], -1.0 / D)
                scr = lnpool.tile([P, D], BF16, tag="xsq", bufs=2)
                nc.scalar.activation(
                    scr[:pt], src[:pt, ti], AF.Square,
                    bias=negmu[:pt, ti:ti + 1], accum_out=varD[:pt, ti:ti + 1])

            def ln_finalize(stats, lo, hi):
                nc.scalar.activation(stats[:, 10 + lo:10 + hi],
                                     stats[:, 5 + lo:5 + hi], AF.Sqrt,
                                     scale=1.0 / D, bias=epsap[:])
                nc.vector.reciprocal(stats[:, 15 + lo:15 + hi],
                                     stats[:, 10 + lo:10 + hi])

            def ln_apply_tiles(stats, src, g_sb, gb_sb, dst_fm, tis):
                # src token-major bf16 -> normalize -> transpose -> scale+shift
                negmu = stats[:, 0:5]
                rsig = stats[:, 15:20]
                for ti in tis:
                    t0, pt = TT[ti]
                    xn = lnpool.tile([P, D], BF16, tag="xn_tm", bufs=3)
                    nc.vector.tensor_scalar(
                        xn[:pt], src[:pt, ti],
                        negmu[:pt, ti:ti + 1], rsig[:pt, ti:ti + 1],
                        OP.add, OP.mult)
                    for kk in range(KK):
                        pst = psA.tile([P, 512], BF16, tag="pA")
                        nc.tensor.transpose(
                            pst[:, :pt], xn[:pt, kk * P:(kk + 1) * P],
                            identb[:pt, :pt])
                        nc.vector.scalar_tensor_tensor(
                            dst_fm[:, kk, t0:t0 + pt], pst[:, :pt],
                            g_sb[:, kk:kk + 1],
                            gb_sb[:, kk:kk + 1].to_broadcast((P, pt)),
                            OP.mult, OP.add)

            def layer_norm_fm(src, g_sb, gb_sb, dst_fm):
                stats = ln_new_stats()
                for ti, (t0, pt) in enumerate(TT):
                    ln_tile_stats(stats, src, ti, pt)
                ln_finalize(stats, 0, 1)
                ln_apply_tiles(stats, src, g_sb, gb_sb, dst_fm, (0,))
                ln_finalize(stats, 1, 5)
                ln_apply_tiles(stats, src, g_sb, gb_sb, dst_fm, (1, 2, 3, 4))

            # ---- per-batch stage emitters ----
            def stage_load(b):
                xb = rpool.tile([P, 5, D], BF16, tag="resid", name=f"xb{b}")
                nc.vector.memset(xb[64:, 4, :], 0.0)
                for ti, (t0, pt) in enumerate(TT):
                    rp = min(pt, S - t0)
                    nc.sync.dma_start(xb[:rp, ti], x_d[b, t0:t0 + rp, :])
                return xb

            def stage_qkv(b, xn_fm):
                q_fm = apool.tile([P, KK, SP], FP8, tag="q")
                k_fm = apool.tile([P, KK, SP], FP8, tag="k")
                v_sb = apool.tile([P, 5, H * VS], FP8, tag="v")

                # q, k: weight-stationary fp8 DoubleRow, feature-major out
                for w_r, bias_sb, dst in ((wq_r, bq_sb, q_fm), (wk_r, bk_sb, k_fm)):
                    for blk in range(2):
                        wb = wpool.tile([P, KK, 512], FP8, tag="wblk")
                        nc.sync.dma_start(wb[:], w_r[:, :, blk * 512:(blk + 1) * 512])
                        for mi in range(4):
                            m = blk * 4 + mi
                            for (q0, qn) in QC:
                                ps = psA.tile([P, 512], F32, tag="pA")
                                for kp in range(4):
                                    nc.tensor.matmul(
                                        ps[:, :qn],
                                        wb[:, 2 * kp:2 * kp + 2, mi * P:(mi + 1) * P],
                                        xn_fm[:, 2 * kp:2 * kp + 2, q0:q0 + qn],
                                        start=(kp == 0), stop=(kp == 3),
                                        perf_mode=DR)
                                nc.scalar.activation(
                                    dst[:, m, q0:q0 + qn], ps[:, :qn],
                                    AF.Identity, scale=1.0 / WQ_SCALE,
                                    bias=bias_sb[:, m:m + 1])

                # v: xn-stationary fp8 DoubleRow, token-major out (16*v in fp8)
                v_hc = v_sb[:].rearrange("p t (h c) -> p t h c", c=VS)
                nc.vector.memset(v_sb[64:, 4, :].bitcast(U8), 0)
                nc.vector.memset(v_hc[:, 0:4, :, 64:65].bitcast(U8), ONE_FP8_BYTE)
                nc.vector.memset(v_hc[0:64, 4, :, 64:65].bitcast(U8), ONE_FP8_BYTE)
                nc.vector.memset(v_hc[64:65, 4, :, 64:65].bitcast(U8), ONE_FP8_BYTE)
                for ci in range(2):
                    wb = wpool.tile([P, KK, 512], FP8, tag="wblk")
                    nc.sync.dma_start(wb[:], wv_r[:, :, ci * 512:(ci + 1) * 512])
                    for ti, (t0, pt) in enumerate(TT):
                        ps = psA.tile([P, 512], F32, tag="pA")
                        for kp in range(4):
                            nc.tensor.matmul(
                                ps[:pt], xn_fm[:, 2 * kp:2 * kp + 2, t0:t0 + pt],
                                wb[:, 2 * kp:2 * kp + 2, :],
                                start=(kp == 0), stop=False, perf_mode=DR)
                        nc.tensor.matmul(
                            ps[:pt], ones8[:, :pt], t_bv[:, ci * 512:(ci + 1) * 512],
                            start=False, stop=True)
                        rp = min(pt, S - t0)
                        nc.scalar.activation(
                            v_hc[:rp, ti, ci * 8:(ci + 1) * 8, 0:64],
                            ps[:rp, :].rearrange("p (h c) -> p h c", c=64),
                            AF.Identity, scale=V_SCALE / WQ_SCALE)
                return q_fm, k_fm, v_sb

            def emit_scores(h, q_fm, k_fm):
                hrow = (h % 2) * 64
                kkh = h // 2
                es = espool.tile([P, 5, SP], FP8, tag="es")
                es4 = es[:, :, 0:578].rearrange("p t (c q) -> p t c q", q=289)
                for kt, (t0, ptk) in enumerate(TT):
                    pg = psS.tile([P, 2, 512], F32, tag="pS")
                    for qi, (q0, qn) in enumerate(QC):
                        nc.tensor.matmul(
                            pg[:ptk, qi, :qn],
                            k_fm[hrow:hrow + 64, kkh, t0:t0 + ptk],
                            q_fm[hrow:hrow + 64, kkh, q0:q0 + qn],
                            start=True, stop=True)
                    nc.scalar.activation(
                        es4[:ptk, kt], pg[:ptk, :, :289],
                        AF.Exp, scale=0.125, bias=nm3[:ptk])
                return es

            def emit_pv(h, es, v_sb, ctx_tm):
                for qt, (q0, qn) in enumerate(TT):
                    pc = psA.tile([P, 512], F32, tag="pA")
                    for pi in range(2):
                        nc.tensor.matmul(
                            pc[:qn, :VS],
                            es[:, 2 * pi:2 * pi + 2, q0:q0 + qn],
                            v_sb[:, 2 * pi:2 * pi + 2, h * VS:(h + 1) * VS],
                            start=(pi == 0), stop=False, perf_mode=DR)
                    nc.tensor.matmul(
                        pc[:qn, :VS], es[:66, 4, q0:q0 + qn],
                        v_sb[:66, 4, h * VS:(h + 1) * VS],
                        start=False, stop=True)
                    rc = rcpool.tile([P, 1], F32, tag="rc")
                    nc.vector.reciprocal(rc[:qn], pc[:qn, 64:65])
                    nc.vector.tensor_scalar_mul(
                        ctx_tm[:qn, qt, h * 64:(h + 1) * 64],
                        pc[:qn, 0:64], rc[:qn])

            def stage_ctx_fm(b, ctx_tm):
                ctx_fm = apool.tile([P, KK, SP], FP8, tag="ctxf")
                for kk in range(KK):
                    for ti, (t0, pt) in enumerate(TT):
                        pst = psA.tile([P, 512], BF16, tag="pA")
                        nc.tensor.transpose(
                            pst[:, :pt], ctx_tm[:pt, ti, kk * P:(kk + 1) * P],
                            identb[:pt, :pt])
                        nc.vector.tensor_copy(ctx_fm[:, kk, t0:t0 + pt],
                                              pst[:, :pt])
                return ctx_fm

            def stage_outproj_ln2(b, ctx_fm, xb):
                # token-major out-projection + residual -> x2 (bf16), LN2 stats
                x2 = rpool.tile([P, 5, D], BF16, tag="resid", name=f"x2{b}")
                stats2 = ln_new_stats()
                for ci in range(2):
                    wb = wpool.tile([P, KK, 512], FP8, tag="wblk")
                    nc.sync.dma_start(wb[:], wo_r[:, :, ci * 512:(ci + 1) * 512])
                    for ti, (t0, pt) in enumerate(TT):
                        ps = psA.tile([P, 512], F32, tag="pA")
                        for kp in range(4):
                            nc.tensor.matmul(
                                ps[:pt], ctx_fm[:, 2 * kp:2 * kp + 2, t0:t0 + pt],
                                wb[:, 2 * kp:2 * kp + 2, :],
                                start=(kp == 0), stop=False, perf_mode=DR)
                        nc.tensor.matmul(
                            ps[:pt], ones8[:, :pt], t_bo[:, ci * 512:(ci + 1) * 512],
                            start=False, stop=True)
                        nc.vector.scalar_tensor_tensor(
                            x2[:pt, ti, ci * 512:(ci + 1) * 512], ps[:pt],
                            1.0 / (CTX_SCALE * WQ_SCALE),
                            xb[:pt, ti, ci * 512:(ci + 1) * 512],
                            OP.mult, OP.add)
                        if ci == 1:
                            ln_tile_stats(stats2, x2, ti, pt)
                return x2, stats2

            def emit_mlp1_chunk(b, m, xn2_fm, h1):
                blk, mi = m // 4, m % 4
                if mi == 0:
                    wb = m1pool.tile([P, KK, 512], BF16, tag="wm1",
                                     name=f"w1_{b}_{blk}")
                    nc.sync.dma_start(wb[:], w1_r[:, :, blk * 512:(blk + 1) * 512])
                    emit_mlp1_chunk.wb = wb
                wb = emit_mlp1_chunk.wb
                ps = psS.tile([P, 2, 512], F32, tag="pS")
                for qi, (q0, qn) in enumerate(QC):
                    for kk in range(KK):
                        nc.tensor.matmul(
                            ps[:, qi, :qn], wb[:, kk, mi * P:(mi + 1) * P],
                            xn2_fm[:, kk, q0:q0 + qn],
                            start=(kk == 0), stop=(kk == KK - 1))
                h14 = h1[:, :, 0:578].rearrange("p t (c q) -> p t c q", q=289)
                nc.scalar.activation(
                    h14[:, m], ps[:, :, :289], _GELU, bias=b1_sb[:, m:m + 1])

            def emit_mlp2_chunk(b, m, h1, mlp_fm, x2):
                wb = m2pool.tile([P, FK, P], BF16, tag="wm2")
                nc.sync.dma_start(wb[:], w2_r[:, :, m * P:(m + 1) * P])
                ps = psS.tile([P, 2, 512], F32, tag="pS")
                for qi, (q0, qn) in enumerate(QC):
                    for kk2 in range(FK):
                        nc.tensor.matmul(
                            ps[:, qi, :qn], wb[:, kk2],
                            h1[:, kk2, q0:q0 + qn],
                            start=(kk2 == 0), stop=(kk2 == FK - 1))
                mf4 = mlp_fm[:, :, 0:578].rearrange("p t (c q) -> p t c q", q=289)
                nc.scalar.activation(
                    mf4[:, m], ps[:, :, :289],
                    AF.Identity, bias=b2_sb[:, m:m + 1])
                # transpose back to token-major, add residual, store
                for ti, (t0, pt) in enumerate(TT):
                    rp = min(pt, S - t0)
                    pst = psA.tile([P, 512], BF16, tag="pA")
                    nc.tensor.transpose(
                        pst[:pt, :P], mlp_fm[:, m, t0:t0 + pt], identb[:])
                    og = opool.tile([P, P], F32, tag="ostg")
                    nc.vector.tensor_tensor(
                        og[:pt], pst[:pt, :P],
                        x2[:pt, ti, m * P:(m + 1) * P], OP.add)
                    nc.sync.dma_start(
                        y_d[b, t0:t0 + rp, m * P:(m + 1) * P], og[:rp])

            # ---- main schedule: MLP(b-1) interleaved under attention(b) ----
            prev = None   # (xn2_fm, x2) of batch b-1
            for b in range(BL + 1):
                if b < BL:
                    xb = stage_load(b)
                    xn_fm = xnpool.tile([P, KK, SP], FP8, tag="xn_fm")
                    layer_norm_fm(xb, g1_sb, gb1_sb, xn_fm)
                    q_fm, k_fm, v_sb = stage_qkv(b, xn_fm)
                    ctx_tm = apool.tile([P, 5, H * 64], BF16, tag="ctxt")

                if prev is not None:
                    pxn2, px2 = prev
                    h1 = mpool.tile([P, FK, SP], BF16, tag="h1")

                for h in range(H):
                    if b < BL:
                        es = emit_scores(h, q_fm, k_fm)
                    if prev is not None:
                        emit_mlp1_chunk(b - 1, 2 * h, pxn2, h1)
                    if b < BL:
                        emit_pv(h, es, v_sb, ctx_tm)
                    if prev is not None:
                        emit_mlp1_chunk(b - 1, 2 * h + 1, pxn2, h1)

                if prev is not None:
                    mlp_fm = mpool.tile([P, KK, SP], BF16, tag="mlp_fm")
                    for m in range(KK):
                        emit_mlp2_chunk(b - 1, m, h1, mlp_fm, px2)
                    prev = None

                if b < BL:
                    ctx_fm = stage_ctx_fm(b, ctx_tm)
                    x2, stats2 = stage_outproj_ln2(b, ctx_fm, xb)
                    xn2_fm = xn2pool.tile([P, KK, SP], BF16, tag="xn2_fm")
                    ln_finalize(stats2, 0, 5)
                    ln_apply_tiles(stats2, x2, g2_sb, gb2_sb, xn2_fm,
                                   (0, 1, 2, 3, 4))
                    prev = (xn2_fm, x2)

    nc.compile()
    return nc


def _get_nc():
    global _NC_CACHE
    if _NC_CACHE is None:
        _NC_CACHE = _build()
    return _NC_CACHE


def kernel(**inputs):
    nc = _get_nc()
    f32 = lambda n: np.ascontiguousarray(np.asarray(inputs[n], dtype=np.float32))

    x = f32("x")
    shared = {
        "wq8": np.ascontiguousarray((f32("wq") * WQ_SCALE).astype(E4NP)),
        "wk8": np.ascontiguousarray((f32("wk") * WQ_SCALE).astype(E4NP)),
        "wv8": np.ascontiguousarray((f32("wv") * WQ_SCALE).astype(E4NP)),
        "wo8": np.ascontiguousarray((f32("wo") * WQ_SCALE).astype(E4NP)),
        "w1b": np.ascontiguousarray(f32("w1").astype(BFNP)),
        "w2b": np.ascontiguousarray(f32("w2").astype(BFNP)),
        "bv8": np.ascontiguousarray((f32("bv") * WQ_SCALE).astype(E4NP)),
        "bo8": np.ascontiguousarray((f32("bo") * CTX_SCALE * WQ_SCALE).astype(E4NP)),
        "bq": f32("bq"), "bk": f32("bk"), "b1": f32("b1"), "b2": f32("b2"),
        "ln1_g": f32("ln1_g"), "ln1_b": f32("ln1_b"),
        "ln2_g": f32("ln2_g"), "ln2_b": f32("ln2_b"),
    }
    in_maps = []
    for i in range(NCORES):
        m = dict(shared)
        m["x"] = np.ascontiguousarray(x[i * BL:(i + 1) * BL].astype(BFNP))
        in_maps.append(m)
    res = bass_utils.run_bass_kernel_spmd(nc, in_maps, core_ids=list(range(NCORES)))
    y = np.concatenate([res.results[i]["y"] for i in range(NCORES)], axis=0)
    return y.astype(np.float32)


# revision 13
# speedup vs baseline: 1.2566x; 1.1506x over previous
"""Trainium2 Bass kernel for a dense transformer block (pre-LN attention + GELU MLP).

Strategy: data-parallel over batch across 8 NeuronCores (2 batches/core, no
collectives).  Mixed precision tuned to the TRN2 PE rates: attention matmuls in
fp8e4 with DoubleRow perf mode (2 K-tiles per instruction at 0.5 cycles/row),
MLP + transposes in bf16 (1.0), residual stream bf16, all accumulation fp32 in
PSUM.  Softmax uses exp(s/8 - 3) without max-subtraction (scores O(1) bounded)
and gets row sums for free from a ones-column in V via an es-stationary PV
matmul.  MLP of batch b-1 is emitted interleaved with attention of batch b so
the scalar-engine Exp stream hides under MLP matmuls.
"""

import numpy as np
import ml_dtypes

import concourse.bass as bass
import concourse.mybir as mybir
import concourse.tile as tile
from concourse import bacc, bass_utils
from concourse.masks import make_identity

# Problem shape (hardcoded per spec nn_Block_58652073394865)
B, S, D, H, F = 16, 577, 1024, 16, 4096
DH = D // H
NCORES = 8
BL = B // NCORES        # batches per core
P = 128
KK = D // P             # 8 chunks of the model dim
FK = F // P             # 32 chunks of the mlp dim
EPS = 1e-6

SP = 592                # token-dim padding: DoubleRow stationary APs need the
                        # slot-pair stride to be 16-byte aligned (592 = 37*16)
TT = [(0, 128), (128, 128), (256, 128), (384, 128), (512, 66)]   # token tiles
QC = [(0, 289), (289, 289)]                                      # moving halves
VS = 65                                                          # 64 v dims + ones col

WQ_SCALE = 32.0         # host-side weight scale into fp8 (w*32 ~ N(0,1))
V_SCALE = 16.0          # v stored as 16*v in fp8
CTX_SCALE = 16.0        # ctx stored as 16*ctx in fp8

F32 = mybir.dt.float32
BF16 = mybir.dt.bfloat16
FP8 = mybir.dt.float8e4
U8 = mybir.dt.uint8
DR = mybir.MatmulPerfMode.DoubleRow
AF = mybir.ActivationFunctionType
OP = mybir.AluOpType

E4NP = ml_dtypes.float8_e4m3
BFNP = ml_dtypes.bfloat16

ONE_FP8_BYTE = int(np.array(1.0, E4NP).view(np.uint8))  # e4m3 encoding of 1.0

_NC_CACHE = None
_GELU = AF.Gelu


def _build():
    nc = bacc.Bacc("TRN2", target_bir_lowering=False, debug=False,
                   num_devices=NCORES)

    x_d = nc.dram_tensor("x", [BL, S, D], BF16, kind="ExternalInput").ap()
    y_d = nc.dram_tensor("y", [BL, S, D], F32, kind="ExternalOutput").ap()
    wq_d = nc.dram_tensor("wq8", [D, D], FP8, kind="ExternalInput").ap()
    wk_d = nc.dram_tensor("wk8", [D, D], FP8, kind="ExternalInput").ap()
    wv_d = nc.dram_tensor("wv8", [D, D], FP8, kind="ExternalInput").ap()
    wo_d = nc.dram_tensor("wo8", [D, D], FP8, kind="ExternalInput").ap()
    w1_d = nc.dram_tensor("w1b", [D, F], BF16, kind="ExternalInput").ap()
    w2_d = nc.dram_tensor("w2b", [F, D], BF16, kind="ExternalInput").ap()
    bv_d = nc.dram_tensor("bv8", [D], FP8, kind="ExternalInput").ap()   # 32*bv
    bo_d = nc.dram_tensor("bo8", [D], FP8, kind="ExternalInput").ap()   # 512*bo
    bq_d = nc.dram_tensor("bq", [D], F32, kind="ExternalInput").ap()
    bk_d = nc.dram_tensor("bk", [D], F32, kind="ExternalInput").ap()
    b1_d = nc.dram_tensor("b1", [F], F32, kind="ExternalInput").ap()
    b2_d = nc.dram_tensor("b2", [D], F32, kind="ExternalInput").ap()
    g1_d = nc.dram_tensor("ln1_g", [D], F32, kind="ExternalInput").ap()
    gb1_d = nc.dram_tensor("ln1_b", [D], F32, kind="ExternalInput").ap()
    g2_d = nc.dram_tensor("ln2_g", [D], F32, kind="ExternalInput").ap()
    gb2_d = nc.dram_tensor("ln2_b", [D], F32, kind="ExternalInput").ap()

    wq_r = wq_d.rearrange("(ko p) d -> p ko d", p=P)
    wk_r = wk_d.rearrange("(ko p) d -> p ko d", p=P)
    wv_r = wv_d.rearrange("(ko p) d -> p ko d", p=P)
    wo_r = wo_d.rearrange("(ko p) d -> p ko d", p=P)
    w1_r = w1_d.rearrange("(ko p) d -> p ko d", p=P)
    w2_r = w2_d.rearrange("(ko p) d -> p ko d", p=P)

    with tile.TileContext(nc) as tc:
        with tc.tile_pool(name="const", bufs=1) as cpool, \
             tc.tile_pool(name="resid", bufs=3) as rpool, \
             tc.tile_pool(name="xnp", bufs=2) as xnpool, \
             tc.tile_pool(name="xn2p", bufs=2) as xn2pool, \
             tc.tile_pool(name="attn", bufs=2) as apool, \
             tc.tile_pool(name="esp", bufs=2) as espool, \
             tc.tile_pool(name="mlp", bufs=1) as mpool, \
             tc.tile_pool(name="wqk", bufs=2) as wpool, \
             tc.tile_pool(name="wm1", bufs=2) as m1pool, \
             tc.tile_pool(name="wm2", bufs=2) as m2pool, \
             tc.tile_pool(name="ostg", bufs=6) as opool, \
             tc.tile_pool(name="lnp", bufs=2) as lnpool, \
             tc.tile_pool(name="rcp", bufs=4) as rcpool, \
             tc.tile_pool(name="psA", bufs=4, space="PSUM") as psA, \
             tc.tile_pool(name="psS", bufs=2, space="PSUM") as psS:

            # ---- constants / small params ----
            cA = cpool.tile([P, 7 * KK + FK], F32, tag="cA")
            bq_sb = cA[:, 0:KK]
            bk_sb = cA[:, KK:2 * KK]
            b2_sb = cA[:, 2 * KK:3 * KK]
            g1_sb = cA[:, 3 * KK:4 * KK]
            gb1_sb = cA[:, 4 * KK:5 * KK]
            g2_sb = cA[:, 5 * KK:6 * KK]
            gb2_sb = cA[:, 6 * KK:7 * KK]
            b1_sb = cA[:, 7 * KK:7 * KK + FK]
            nc.sync.dma_start(bq_sb, bq_d.rearrange("(m p) -> p m", p=P))
            nc.sync.dma_start(bk_sb, bk_d.rearrange("(m p) -> p m", p=P))
            nc.sync.dma_start(b2_sb, b2_d.rearrange("(m p) -> p m", p=P))
            nc.sync.dma_start(g1_sb, g1_d.rearrange("(c p) -> p c", p=P))
            nc.sync.dma_start(gb1_sb, gb1_d.rearrange("(c p) -> p c", p=P))
            nc.sync.dma_start(g2_sb, g2_d.rearrange("(c p) -> p c", p=P))
            nc.sync.dma_start(gb2_sb, gb2_d.rearrange("(c p) -> p c", p=P))
            nc.sync.dma_start(b1_sb, b1_d.rearrange("(m p) -> p m", p=P))

            cB = cpool.tile([P, P + 2], F32, tag="cB")
            identf = cB[:, 0:P]
            epsap = cB[:, P:P + 1]
            nm3 = cB[:, P + 1:P + 2]
            make_identity(nc, identf)
            nc.vector.memset(epsap, EPS)
            nc.vector.memset(nm3, -3.0)

            identb = cpool.tile([P, P], BF16, tag="identb")
            nc.vector.tensor_copy(identb[:], identf)
            ident8 = cpool.tile([P, P], FP8, tag="ident8")
            nc.vector.tensor_copy(ident8[:], identf)

            # fp8 bias rows for the K=1 bias matmuls (token-major outputs)
            cD = cpool.tile([1, 2 * D + P], FP8, tag="cD")
            t_bv = cD[:, 0:D]
            t_bo = cD[:, D:2 * D]
            ones8 = cD[:, 2 * D:2 * D + P]
            nc.sync.dma_start(t_bv, bv_d[None, :])
            nc.sync.dma_start(t_bo, bo_d[None, :])
            nc.vector.memset(ones8.bitcast(U8), ONE_FP8_BYTE)

            # ---- layernorm helpers (token-major stats, feature-major out) ----
            def ln_new_stats():
                stats = lnpool.tile([P, 20], F32, tag="stats")
                nc.vector.memset(stats[:, 0:5], 0.0)
                nc.vector.memset(stats[:, 5:10], 1.0)
                return stats

            def ln_tile_stats(stats, src, ti, pt):
                negmu = stats[:, 0:5]
                varD = stats[:, 5:10]
                nc.vector.tensor_reduce(
                    negmu[:pt, ti:ti + 1], src[:pt, ti],
                    mybir.AxisListType.X, OP.add)
                nc.vector.tensor_scalar_mul(
                    negmu[:pt, ti:ti + 1], negmu[:pt, ti:ti + 1], -1.0 / D)
                scr = lnpool.tile([P, D], BF16, tag="xsq", bufs=2)
                nc.scalar.activation(
                    scr[:pt], src[:pt, ti], AF.Square,
                    bias=negmu[:pt, ti:ti + 1], accum_out=varD[:pt, ti:ti + 1])

            def ln_finalize(stats, lo, hi):
                nc.scalar.activation(stats[:, 10 + lo:10 + hi],
                                     stats[:, 5 + lo:5 + hi], AF.Sqrt,
                                     scale=1.0 / D, bias=epsap[:])
                nc.vector.reciprocal(stats[:, 15 + lo:15 + hi],
                                     stats[:, 10 + lo:10 + hi])

            def ln_apply_tiles(stats, src, g_sb, gb_sb, dst_fm, tis):
                # src token-major bf16 -> normalize -> transpose -> scale+shift
                negmu = stats[:, 0:5]
                rsig = stats[:, 15:20]
                for ti in tis:
                    t0, pt = TT[ti]
                    xn = lnpool.tile([P, D], BF16, tag="xn_tm", bufs=3)
                    nc.vector.tensor_scalar(
                        xn[:pt], src[:pt, ti],
                        negmu[:pt, ti:ti + 1], rsig[:pt, ti:ti + 1],
                        OP.add, OP.mult)
                    for kk in range(KK):
                        pst = psA.tile([P, 512], BF16, tag="pA")
                        nc.tensor.transpose(
                            pst[:, :pt], xn[:pt, kk * P:(kk + 1) * P],
                            identb[:pt, :pt])
                        nc.vector.scalar_tensor_tensor(
                            dst_fm[:, kk, t0:t0 + pt], pst[:, :pt],
                            g_sb[:, kk:kk + 1],
                            gb_sb[:, kk:kk + 1].to_broadcast((P, pt)),
                            OP.mult, OP.add)

            def layer_norm_fm(src, g_sb, gb_sb, dst_fm):
                stats = ln_new_stats()
                for ti, (t0, pt) in enumerate(TT):
                    ln_tile_stats(stats, src, ti, pt)
                ln_finalize(stats, 0, 1)
                ln_apply_tiles(stats, src, g_sb, gb_sb, dst_fm, (0,))
                ln_finalize(stats, 1, 5)
                ln_apply_tiles(stats, src, g_sb, gb_sb, dst_fm, (1, 2, 3, 4))

            # ---- per-batch stage emitters ----
            def stage_load(b):
                xb = rpool.tile([P, 5, D], BF16, tag="resid", name=f"xb{b}")
                nc.vector.memset(xb[64:, 4, :], 0.0)
                for ti, (t0, pt) in enumerate(TT):
                    rp = min(pt, S - t0)
                    nc.sync.dma_start(xb[:rp, ti], x_d[b, t0:t0 + rp, :])
                return xb

            def stage_qkv(b, xn_fm):
                q_fm = apool.tile([P, KK, SP], FP8, tag="q")
                k_fm = apool.tile([P, KK, SP], FP8, tag="k")
                v_sb = apool.tile([P, 5, H * VS], FP8, tag="v")

                # q, k: weight-stationary fp8 DoubleRow, feature-major out
                for w_r, bias_sb, dst in ((wq_r, bq_sb, q_fm), (wk_r, bk_sb, k_fm)):
                    for blk in range(2):
                        wb = wpool.tile([P, KK, 512], FP8, tag="wblk")
                        nc.sync.dma_start(wb[:], w_r[:, :, blk * 512:(blk + 1) * 512])
                        for mi in range(4):
                            m = blk * 4 + mi
                            for (q0, qn) in QC:
                                ps = psA.tile([P, 512], F32, tag="pA")
                                for kp in range(4):
                                    nc.tensor.matmul(
                                        ps[:, :qn],
                                        wb[:, 2 * kp:2 * kp + 2, mi * P:(mi + 1) * P],
                                        xn_fm[:, 2 * kp:2 * kp + 2, q0:q0 + qn],
                                        start=(kp == 0), stop=(kp == 3),
                                        perf_mode=DR)
                                nc.vector.tensor_scalar(
                                    dst[:, m, q0:q0 + qn], ps[:, :qn],
                                    1.0 / WQ_SCALE, bias_sb[:, m:m + 1],
                                    OP.mult, OP.add)

                # v: xn-stationary fp8 DoubleRow, token-major out (16*v in fp8)
                v_hc = v_sb[:].rearrange("p t (h c) -> p t h c", c=VS)
                nc.vector.memset(v_sb[64:, 4, :].bitcast(U8), 0)
                nc.vector.memset(v_hc[:, 0:4, :, 64:65].bitcast(U8), ONE_FP8_BYTE)
                nc.vector.memset(v_hc[0:64, 4, :, 64:65].bitcast(U8), ONE_FP8_BYTE)
                nc.vector.memset(v_hc[64:65, 4, :, 64:65].bitcast(U8), ONE_FP8_BYTE)
                for ci in range(2):
                    wb = wpool.tile([P, KK, 512], FP8, tag="wblk")
                    nc.sync.dma_start(wb[:], wv_r[:, :, ci * 512:(ci + 1) * 512])
                    for ti, (t0, pt) in enumerate(TT):
                        ps = psA.tile([P, 512], F32, tag="pA")
                        for kp in range(4):
                            nc.tensor.matmul(
                                ps[:pt], xn_fm[:, 2 * kp:2 * kp + 2, t0:t0 + pt],
                                wb[:, 2 * kp:2 * kp + 2, :],
                                start=(kp == 0), stop=False, perf_mode=DR)
                        nc.tensor.matmul(
                            ps[:pt], ones8[:, :pt], t_bv[:, ci * 512:(ci + 1) * 512],
                            start=False, stop=True)
                        rp = min(pt, S - t0)
                        nc.vector.tensor_scalar_mul(
                            v_hc[:rp, ti, ci * 8:(ci + 1) * 8, 0:64],
                            ps[:rp, :].rearrange("p (h c) -> p h c", c=64),
                            V_SCALE / WQ_SCALE)
                return q_fm, k_fm, v_sb

            def emit_scores(h, q_fm, k_fm):
                hrow = (h % 2) * 64
                kkh = h // 2
                es = espool.tile([P, 5, SP], FP8, tag="es")
                es4 = es[:, :, 0:578].rearrange("p t (c q) -> p t c q", q=289)
                for kt, (t0, ptk) in enumerate(TT):
                    pg = psS.tile([P, 2, 512], F32, tag="pS")
                    for qi, (q0, qn) in enumerate(QC):
                        nc.tensor.matmul(
                            pg[:ptk, qi, :qn],
                            k_fm[hrow:hrow + 64, kkh, t0:t0 + ptk],
                            q_fm[hrow:hrow + 64, kkh, q0:q0 + qn],
                            start=True, stop=True)
                    nc.scalar.activation(
                        es4[:ptk, kt], pg[:ptk, :, :289],
                        AF.Exp, scale=0.125, bias=nm3[:ptk])
                return es

            def emit_pv(h, es, v_sb, ctx_tm):
                for qt, (q0, qn) in enumerate(TT):
                    pc = psA.tile([P, 512], F32, tag="pA")
                    for pi in range(2):
                        nc.tensor.matmul(
                            pc[:qn, :VS],
                            es[:, 2 * pi:2 * pi + 2, q0:q0 + qn],
                            v_sb[:, 2 * pi:2 * pi + 2, h * VS:(h + 1) * VS],
                            start=(pi == 0), stop=False, perf_mode=DR)
                    nc.tensor.matmul(
                        pc[:qn, :VS], es[:66, 4, q0:q0 + qn],
                        v_sb[:66, 4, h * VS:(h + 1) * VS],
                        start=False, stop=True)
                    rc = rcpool.tile([P, 1], F32, tag="rc")
                    nc.vector.reciprocal(rc[:qn], pc[:qn, 64:65])
                    nc.vector.tensor_scalar_mul(
                        ctx_tm[:qn, qt, h * 64:(h + 1) * 64],
                        pc[:qn, 0:64], rc[:qn])

            def stage_ctx_fm(b, ctx_tm):
                ctx_fm = apool.tile([P, KK, SP], FP8, tag="ctxf", bufs=1)
                for kk in range(KK):
                    for ti, (t0, pt) in enumerate(TT):
                        pst = psA.tile([P, 512], BF16, tag="pA")
                        nc.tensor.transpose(
                            pst[:, :pt], ctx_tm[:pt, ti, kk * P:(kk + 1) * P],
                            identb[:pt, :pt])
                        nc.vector.tensor_copy(ctx_fm[:, kk, t0:t0 + pt],
                                              pst[:, :pt])
                return ctx_fm

            def stage_outproj_ln2(b, ctx_fm, xb):
                # token-major out-projection + residual -> x2 (bf16), LN2 stats
                x2 = rpool.tile([P, 5, D], BF16, tag="resid", name=f"x2{b}")
                stats2 = ln_new_stats()
                for ci in range(2):
                    wb = wpool.tile([P, KK, 512], FP8, tag="wblk")
                    nc.sync.dma_start(wb[:], wo_r[:, :, ci * 512:(ci + 1) * 512])
                    for ti, (t0, pt) in enumerate(TT):
                        ps = psA.tile([P, 512], F32, tag="pA")
                        for kp in range(4):
                            nc.tensor.matmul(
                                ps[:pt], ctx_fm[:, 2 * kp:2 * kp + 2, t0:t0 + pt],
                                wb[:, 2 * kp:2 * kp + 2, :],
                                start=(kp == 0), stop=False, perf_mode=DR)
                        nc.tensor.matmul(
                            ps[:pt], ones8[:, :pt], t_bo[:, ci * 512:(ci + 1) * 512],
                            start=False, stop=True)
                        nc.vector.scalar_tensor_tensor(
                            x2[:pt, ti, ci * 512:(ci + 1) * 512], ps[:pt],
                            1.0 / (CTX_SCALE * WQ_SCALE),
                            xb[:pt, ti, ci * 512:(ci + 1) * 512],
                            OP.mult, OP.add)
                        if ci == 1:
                            ln_tile_stats(stats2, x2, ti, pt)
                return x2, stats2

            def emit_mlp1_chunk(b, m, xn2_fm, h1):
                blk, mi = m // 4, m % 4
                if mi == 0:
                    wb = m1pool.tile([P, KK, 512], BF16, tag="wm1",
                                     name=f"w1_{b}_{blk}")
                    nc.sync.dma_start(wb[:], w1_r[:, :, blk * 512:(blk + 1) * 512])
                    emit_mlp1_chunk.wb = wb
                wb = emit_mlp1_chunk.wb
                ps = psS.tile([P, 2, 512], F32, tag="pS")
                for qi, (q0, qn) in enumerate(QC):
                    for kk in range(KK):
                        nc.tensor.matmul(
                            ps[:, qi, :qn], wb[:, kk, mi * P:(mi + 1) * P],
                            xn2_fm[:, kk, q0:q0 + qn],
                            start=(kk == 0), stop=(kk == KK - 1))
                h14 = h1[:, :, 0:578].rearrange("p t (c q) -> p t c q", q=289)
                nc.scalar.activation(
                    h14[:, m], ps[:, :, :289], _GELU, bias=b1_sb[:, m:m + 1])

            def emit_mlp2_group(b, idx, h1, mlp_fm, x2):
                # one (m-chunk, token-half) accumulation group of h1 @ w2
                m, qi = idx // 2, idx % 2
                if qi == 0:
                    wb = m2pool.tile([P, FK, P], BF16, tag="wm2")
                    nc.sync.dma_start(wb[:], w2_r[:, :, m * P:(m + 1) * P])
                    emit_mlp2_group.wb = wb
                wb = emit_mlp2_group.wb
                q0, qn = QC[qi]
                ps = psA.tile([P, 512], F32, tag="pA")
                for kk2 in range(FK):
                    nc.tensor.matmul(
                        ps[:, :qn], wb[:, kk2], h1[:, kk2, q0:q0 + qn],
                        start=(kk2 == 0), stop=(kk2 == FK - 1))
                nc.scalar.activation(
                    mlp_fm[:, m, q0:q0 + qn], ps[:, :qn],
                    AF.Identity, bias=b2_sb[:, m:m + 1])
                if qi == 1:
                    # transpose back to token-major, add residual, store
                    for ti, (t0, pt) in enumerate(TT):
                        rp = min(pt, S - t0)
                        pst = psA.tile([P, 512], BF16, tag="pA")
                        nc.tensor.transpose(
                            pst[:pt, :P], mlp_fm[:, m, t0:t0 + pt], identb[:])
                        og = opool.tile([P, P], F32, tag="ostg")
                        nc.vector.tensor_tensor(
                            og[:pt], pst[:pt, :P],
                            x2[:pt, ti, m * P:(m + 1) * P], OP.add)
                        nc.sync.dma_start(
                            y_d[b, t0:t0 + rp, m * P:(m + 1) * P], og[:rp])

            # ---- batch prep (load + LN1 + QKV), splittable into units so it
            # can be spread under the previous batch's attention ----
            bstate = {}

            def prep_units(b):
                st = {}
                bstate[b] = st

                def u_load():
                    st["xb"] = stage_load(b)
                    st["stats"] = ln_new_stats()
                    for ti, (t0, pt) in enumerate(TT):
                        ln_tile_stats(st["stats"], st["xb"], ti, pt)

                def u_ln():
                    xn_fm = xnpool.tile([P, KK, SP], FP8, tag="xn_fm", bufs=1)
                    st["xn_fm"] = xn_fm
                    ln_finalize(st["stats"], 0, 5)
                    ln_apply_tiles(st["stats"], st["xb"], g1_sb, gb1_sb,
                                   xn_fm, (0, 1, 2, 3, 4))

                def u_qkv():
                    st["q"], st["k"], st["v"] = stage_qkv(b, st["xn_fm"])

                return [u_load, u_ln, u_qkv]

            # ---- main schedule ----
            # slot b: [MLP1(b-1) gelu-run] ; [attention(b) || MLP2(b-1) ||
            # prep(b+1)] ; [ctxT/outproj/LN2(b)]
            prev = None   # (xn2_fm, x2) of batch b-1
            for u in prep_units(0):
                u()
            for slot in range(BL + 1):
                b = slot if slot < BL else None
                pb = slot - 1 if slot >= 1 else None

                # phase 1: MLP1(pb) — contiguous gelu run on Act
                if pb is not None:
                    pxn2, px2 = prev
                    h1 = mpool.tile([P, FK, SP], BF16, tag="h1")
                    for m in range(FK):
                        emit_mlp1_chunk(pb, m, pxn2, h1)
                    mlp_fm = mpool.tile([P, KK, SP], BF16, tag="mlp_fm")

                # phase 2: attention(b) || MLP2(pb) || prep(b+1)
                if b is not None:
                    st = bstate[b]
                    q_fm, k_fm, v_sb = st["q"], st["k"], st["v"]
                    ctx_tm = apool.tile([P, 5, H * 64], BF16, tag="ctxt",
                                        bufs=1)
                    units = prep_units(b + 1) if b + 1 < BL else []
                    for h in range(H):
                        es = emit_scores(h, q_fm, k_fm)
                        if pb is not None:
                            emit_mlp2_group(pb, h, h1, mlp_fm, px2)
                        emit_pv(h, es, v_sb, ctx_tm)
                        if units and h % 5 == 4:
                            units.pop(0)()
                    for u in units:
                        u()
                elif pb is not None:
                    for idx in range(H):
                        emit_mlp2_group(pb, idx, h1, mlp_fm, px2)

                # phase 3: ctx transpose + out-projection + LN2
                if b is not None:
                    ctx_fm = stage_ctx_fm(b, ctx_tm)
                    x2, stats2 = stage_outproj_ln2(b, ctx_fm, st["xb"])
                    xn2_fm = xn2pool.tile([P, KK, SP], BF16, tag="xn2_fm",
                                          bufs=1)
                    ln_finalize(stats2, 0, 5)
                    ln_apply_tiles(stats2, x2, g2_sb, gb2_sb, xn2_fm,
                                   (0, 1, 2, 3, 4))
                    prev = (xn2_fm, x2)

    nc.compile()
    return nc


def _get_nc():
    global _NC_CACHE
    if _NC_CACHE is None:
        _NC_CACHE = _build()
    return _NC_CACHE


def kernel(**inputs):
    nc = _get_nc()
    f32 = lambda n: np.ascontiguousarray(np.asarray(inputs[n], dtype=np.float32))

    x = f32("x")
    shared = {
        "wq8": np.ascontiguousarray((f32("wq") * WQ_SCALE).astype(E4NP)),
        "wk8": np.ascontiguousarray((f32("wk") * WQ_SCALE).astype(E4NP)),
        "wv8": np.ascontiguousarray((f32("wv") * WQ_SCALE).astype(E4NP)),
        "wo8": np.ascontiguousarray((f32("wo") * WQ_SCALE).astype(E4NP)),
        "w1b": np.ascontiguousarray(f32("w1").astype(BFNP)),
        "w2b": np.ascontiguousarray(f32("w2").astype(BFNP)),
        "bv8": np.ascontiguousarray((f32("bv") * WQ_SCALE).astype(E4NP)),
        "bo8": np.ascontiguousarray((f32("bo") * CTX_SCALE * WQ_SCALE).astype(E4NP)),
        "bq": f32("bq"), "bk": f32("bk"), "b1": f32("b1"), "b2": f32("b2"),
        "ln1_g": f32("ln1_g"), "ln1_b": f32("ln1_b"),
        "ln2_g": f32("ln2_g"), "ln2_b": f32("ln2_b"),
    }
    in_maps = []
    for i in range(NCORES):
        m = dict(shared)
        m["x"] = np.ascontiguousarray(x[i * BL:(i + 1) * BL].astype(BFNP))
        in_maps.append(m)
    res = bass_utils.run_bass_kernel_spmd(nc, in_maps, core_ids=list(range(NCORES)))
    y = np.concatenate([res.results[i]["y"] for i in range(NCORES)], axis=0)
    return y.astype(np.float32)


# revision 18
# speedup vs baseline: 1.3136x; 1.0454x over previous
"""Trainium2 Bass kernel for a dense transformer block (pre-LN attention + GELU MLP).

Strategy: data-parallel over batch across 8 NeuronCores (2 batches/core, no
collectives).  Mixed precision tuned to the TRN2 PE rates: attention matmuls in
fp8e4 with DoubleRow perf mode (2 K-tiles per instruction at 0.5 cycles/row),
MLP + transposes in bf16 (1.0), residual stream bf16, all accumulation fp32 in
PSUM.  Softmax uses exp(s/8 - 3) without max-subtraction (scores O(1) bounded)
and gets row sums for free from a ones-column in V via an es-stationary PV
matmul.  MLP of batch b-1 is emitted interleaved with attention of batch b so
the scalar-engine Exp stream hides under MLP matmuls.
"""

import numpy as np
import ml_dtypes

import concourse.bass as bass
import concourse.mybir as mybir
import concourse.tile as tile
from concourse import bacc, bass_utils
from concourse.masks import make_identity

# Problem shape (hardcoded per spec nn_Block_58652073394865)
B, S, D, H, F = 16, 577, 1024, 16, 4096
DH = D // H
NCORES = 8
BL = B // NCORES        # batches per core
P = 128
KK = D // P             # 8 chunks of the model dim
FK = F // P             # 32 chunks of the mlp dim
EPS = 1e-6

SP = 592                # token-dim padding: DoubleRow stationary APs need the
                        # slot-pair stride to be 16-byte aligned (592 = 37*16)
TT = [(0, 128), (128, 128), (256, 128), (384, 128), (512, 66)]   # token tiles
QC = [(0, 289), (289, 289)]                                      # moving halves
VS = 65                                                          # 64 v dims + ones col

WQ_SCALE = 32.0         # host-side weight scale into fp8 (w*32 ~ N(0,1))
V_SCALE = 16.0          # v stored as 16*v in fp8
CTX_SCALE = 16.0        # ctx stored as 16*ctx in fp8

F32 = mybir.dt.float32
BF16 = mybir.dt.bfloat16
FP8 = mybir.dt.float8e4
U8 = mybir.dt.uint8
DR = mybir.MatmulPerfMode.DoubleRow
AF = mybir.ActivationFunctionType
OP = mybir.AluOpType

E4NP = ml_dtypes.float8_e4m3
BFNP = ml_dtypes.bfloat16

ONE_FP8_BYTE = int(np.array(1.0, E4NP).view(np.uint8))  # e4m3 encoding of 1.0

_NC_CACHE = None
_GELU = AF.Gelu


def _build():
    nc = bacc.Bacc("TRN2", target_bir_lowering=False, debug=False,
                   num_devices=NCORES)

    x_d = nc.dram_tensor("x", [BL, S, D], BF16, kind="ExternalInput").ap()
    y_d = nc.dram_tensor("y", [BL, S, D], F32, kind="ExternalOutput").ap()
    wq_d = nc.dram_tensor("wq8", [D, D], FP8, kind="ExternalInput").ap()
    wk_d = nc.dram_tensor("wk8", [D, D], FP8, kind="ExternalInput").ap()
    wv_d = nc.dram_tensor("wv8", [D, D], FP8, kind="ExternalInput").ap()
    wo_d = nc.dram_tensor("wo8", [D, D], FP8, kind="ExternalInput").ap()
    w1_d = nc.dram_tensor("w1b", [D, F], BF16, kind="ExternalInput").ap()
    w2_d = nc.dram_tensor("w2b", [F, D], BF16, kind="ExternalInput").ap()
    # packed small params: [bq bk b2 g1 gb1 g2 gb2 | b1] column blocks of KK
    pf_d = nc.dram_tensor("params_f32", [P, 7 * KK + FK], F32,
                          kind="ExternalInput").ap()
    # packed fp8 row: [32*bv | 512*bo | ones]
    p8_d = nc.dram_tensor("params_fp8", [1, 2 * D + P], FP8,
                          kind="ExternalInput").ap()

    wq_r = wq_d.rearrange("(ko p) d -> p ko d", p=P)
    wk_r = wk_d.rearrange("(ko p) d -> p ko d", p=P)
    wv_r = wv_d.rearrange("(ko p) d -> p ko d", p=P)
    wo_r = wo_d.rearrange("(ko p) d -> p ko d", p=P)
    w1_r = w1_d.rearrange("(ko p) d -> p ko d", p=P)
    w2_r = w2_d.rearrange("(ko p) d -> p ko d", p=P)

    with tile.TileContext(nc) as tc:
        with tc.tile_pool(name="const", bufs=1) as cpool, \
             tc.tile_pool(name="resid", bufs=3) as rpool, \
             tc.tile_pool(name="xnp", bufs=2) as xnpool, \
             tc.tile_pool(name="xn2p", bufs=2) as xn2pool, \
             tc.tile_pool(name="attn", bufs=2) as apool, \
             tc.tile_pool(name="esp", bufs=2) as espool, \
             tc.tile_pool(name="mlp", bufs=1) as mpool, \
             tc.tile_pool(name="wqk", bufs=2) as wpool, \
             tc.tile_pool(name="wm1", bufs=2) as m1pool, \
             tc.tile_pool(name="wm2", bufs=2) as m2pool, \
             tc.tile_pool(name="ostg", bufs=6) as opool, \
             tc.tile_pool(name="lnp", bufs=2) as lnpool, \
             tc.tile_pool(name="rcp", bufs=4) as rcpool, \
             tc.tile_pool(name="psA", bufs=4, space="PSUM") as psA, \
             tc.tile_pool(name="psS", bufs=2, space="PSUM") as psS:

            # ---- constants / small params ----
            cA = cpool.tile([P, 7 * KK + FK], F32, tag="cA")
            bq_sb = cA[:, 0:KK]
            bk_sb = cA[:, KK:2 * KK]
            b2_sb = cA[:, 2 * KK:3 * KK]
            g1_sb = cA[:, 3 * KK:4 * KK]
            gb1_sb = cA[:, 4 * KK:5 * KK]
            g2_sb = cA[:, 5 * KK:6 * KK]
            gb2_sb = cA[:, 6 * KK:7 * KK]
            b1_sb = cA[:, 7 * KK:7 * KK + FK]
            nc.sync.dma_start(cA[:], pf_d)

            cB = cpool.tile([P, P + 2], F32, tag="cB")
            identf = cB[:, 0:P]
            epsap = cB[:, P:P + 1]
            nm3 = cB[:, P + 1:P + 2]
            make_identity(nc, identf)
            nc.vector.memset(epsap, EPS)
            nc.vector.memset(nm3, -3.0)

            identb = cpool.tile([P, P], BF16, tag="identb")
            nc.vector.tensor_copy(identb[:], identf)
            ident8 = cpool.tile([P, P], FP8, tag="ident8")
            nc.vector.tensor_copy(ident8[:], identf)

            # fp8 bias rows for the K=1 bias matmuls (token-major outputs)
            cD = cpool.tile([1, 2 * D + P], FP8, tag="cD")
            t_bv = cD[:, 0:D]
            t_bo = cD[:, D:2 * D]
            ones8 = cD[:, 2 * D:2 * D + P]
            nc.sync.dma_start(cD[:], p8_d)

            # ---- layernorm helpers (token-major stats, feature-major out) ----
            def ln_new_stats():
                stats = lnpool.tile([P, 20], F32, tag="stats")
                nc.vector.memset(stats[:, 0:5], 0.0)
                nc.vector.memset(stats[:, 5:10], 1.0)
                return stats

            def ln_tile_stats(stats, src, ti, pt):
                negmu = stats[:, 0:5]
                varD = stats[:, 5:10]
                nc.vector.tensor_reduce(
                    negmu[:pt, ti:ti + 1], src[:pt, ti],
                    mybir.AxisListType.X, OP.add)
                nc.vector.tensor_scalar_mul(
                    negmu[:pt, ti:ti + 1], negmu[:pt, ti:ti + 1], -1.0 / D)
                scr = lnpool.tile([P, D], BF16, tag="xsq", bufs=2)
                nc.scalar.activation(
                    scr[:pt], src[:pt, ti], AF.Square,
                    bias=negmu[:pt, ti:ti + 1], accum_out=varD[:pt, ti:ti + 1])

            def ln_finalize(stats, lo, hi):
                nc.scalar.activation(stats[:, 10 + lo:10 + hi],
                                     stats[:, 5 + lo:5 + hi], AF.Sqrt,
                                     scale=1.0 / D, bias=epsap[:])
                nc.vector.reciprocal(stats[:, 15 + lo:15 + hi],
                                     stats[:, 10 + lo:10 + hi])

            def ln_apply_tiles(stats, src, g_sb, gb_sb, dst_fm, tis):
                # src token-major bf16 -> normalize -> transpose -> scale+shift
                negmu = stats[:, 0:5]
                rsig = stats[:, 15:20]
                for ti in tis:
                    t0, pt = TT[ti]
                    xn = lnpool.tile([P, D], BF16, tag="xn_tm", bufs=3)
                    nc.vector.tensor_scalar(
                        xn[:pt], src[:pt, ti],
                        negmu[:pt, ti:ti + 1], rsig[:pt, ti:ti + 1],
                        OP.add, OP.mult)
                    for kk in range(KK):
                        pst = psA.tile([P, 512], BF16, tag="pA")
                        nc.tensor.transpose(
                            pst[:, :pt], xn[:pt, kk * P:(kk + 1) * P],
                            identb[:pt, :pt])
                        nc.vector.scalar_tensor_tensor(
                            dst_fm[:, kk, t0:t0 + pt], pst[:, :pt],
                            g_sb[:, kk:kk + 1],
                            gb_sb[:, kk:kk + 1].to_broadcast((P, pt)),
                            OP.mult, OP.add)

            def layer_norm_fm(src, g_sb, gb_sb, dst_fm):
                stats = ln_new_stats()
                for ti, (t0, pt) in enumerate(TT):
                    ln_tile_stats(stats, src, ti, pt)
                ln_finalize(stats, 0, 1)
                ln_apply_tiles(stats, src, g_sb, gb_sb, dst_fm, (0,))
                ln_finalize(stats, 1, 5)
                ln_apply_tiles(stats, src, g_sb, gb_sb, dst_fm, (1, 2, 3, 4))

            # ---- per-batch stage emitters ----
            def stage_load(b):
                xb = rpool.tile([P, 5, D], BF16, tag="resid", name=f"xb{b}")
                nc.vector.memset(xb[64:, 4, :], 0.0)
                for ti, (t0, pt) in enumerate(TT):
                    rp = min(pt, S - t0)
                    nc.sync.dma_start(xb[:rp, ti], x_d[b, t0:t0 + rp, :])
                return xb

            def emit_qk_block(st, which, blk):
                w_r, bias_sb = ((wq_r, bq_sb) if which == "q" else (wk_r, bk_sb))
                if which not in st:
                    st[which] = apool.tile([P, KK, SP], FP8, tag=which,
                                           name=f"{which}_fm")
                dst = st[which]
                xn_fm = st["xn_fm"]
                wb = wpool.tile([P, KK, 512], FP8, tag="wblk")
                nc.sync.dma_start(wb[:], w_r[:, :, blk * 512:(blk + 1) * 512])
                for mi in range(4):
                    m = blk * 4 + mi
                    for (q0, qn) in QC:
                        ps = psA.tile([P, 512], F32, tag="pA")
                        for kp in range(4):
                            nc.tensor.matmul(
                                ps[:, :qn],
                                wb[:, 2 * kp:2 * kp + 2, mi * P:(mi + 1) * P],
                                xn_fm[:, 2 * kp:2 * kp + 2, q0:q0 + qn],
                                start=(kp == 0), stop=(kp == 3),
                                perf_mode=DR)
                        nc.vector.tensor_scalar(
                            dst[:, m, q0:q0 + qn], ps[:, :qn],
                            1.0 / WQ_SCALE, bias_sb[:, m:m + 1],
                            OP.mult, OP.add)

            def emit_v_block(st, ci):
                xn_fm = st["xn_fm"]
                if "v" not in st:
                    v_sb = apool.tile([P, 5, H * VS], FP8, tag="v",
                                      name="v_sb")
                    st["v"] = v_sb
                    v_hc = v_sb[:].rearrange("p t (h c) -> p t h c", c=VS)
                    st["v_hc"] = v_hc
                    nc.vector.memset(v_sb[64:, 4, :].bitcast(U8), 0)
                    nc.vector.memset(v_hc[:, 0:4, :, 64:65].bitcast(U8), ONE_FP8_BYTE)
                    nc.vector.memset(v_hc[0:64, 4, :, 64:65].bitcast(U8), ONE_FP8_BYTE)
                    nc.vector.memset(v_hc[64:65, 4, :, 64:65].bitcast(U8), ONE_FP8_BYTE)
                v_hc = st["v_hc"]
                wb = wpool.tile([P, KK, 512], FP8, tag="wblk")
                nc.sync.dma_start(wb[:], wv_r[:, :, ci * 512:(ci + 1) * 512])
                for ti, (t0, pt) in enumerate(TT):
                    ps = psA.tile([P, 512], F32, tag="pA")
                    for kp in range(4):
                        nc.tensor.matmul(
                            ps[:pt], xn_fm[:, 2 * kp:2 * kp + 2, t0:t0 + pt],
                            wb[:, 2 * kp:2 * kp + 2, :],
                            start=(kp == 0), stop=False, perf_mode=DR)
                    nc.tensor.matmul(
                        ps[:pt], ones8[:, :pt], t_bv[:, ci * 512:(ci + 1) * 512],
                        start=False, stop=True)
                    rp = min(pt, S - t0)
                    nc.vector.tensor_scalar_mul(
                        v_hc[:rp, ti, ci * 8:(ci + 1) * 8, 0:64],
                        ps[:rp, :].rearrange("p (h c) -> p h c", c=64),
                        V_SCALE / WQ_SCALE)

            def emit_scores(h, q_fm, k_fm):
                hrow = (h % 2) * 64
                kkh = h // 2
                es = espool.tile([P, 5, SP], FP8, tag="es")
                es4 = es[:, :, 0:578].rearrange("p t (c q) -> p t c q", q=289)
                for kt, (t0, ptk) in enumerate(TT):
                    pg = psS.tile([P, 2, 512], F32, tag="pS")
                    for qi, (q0, qn) in enumerate(QC):
                        nc.tensor.matmul(
                            pg[:ptk, qi, :qn],
                            k_fm[hrow:hrow + 64, kkh, t0:t0 + ptk],
                            q_fm[hrow:hrow + 64, kkh, q0:q0 + qn],
                            start=True, stop=True)
                    nc.scalar.activation(
                        es4[:ptk, kt], pg[:ptk, :, :289],
                        AF.Exp, scale=0.125, bias=nm3[:ptk])
                return es

            def emit_pv(h, es, v_sb, ctx_tm):
                for qt, (q0, qn) in enumerate(TT):
                    pc = psA.tile([P, 512], F32, tag="pA")
                    for pi in range(2):
                        nc.tensor.matmul(
                            pc[:qn, :VS],
                            es[:, 2 * pi:2 * pi + 2, q0:q0 + qn],
                            v_sb[:, 2 * pi:2 * pi + 2, h * VS:(h + 1) * VS],
                            start=(pi == 0), stop=False, perf_mode=DR)
                    nc.tensor.matmul(
                        pc[:qn, :VS], es[:66, 4, q0:q0 + qn],
                        v_sb[:66, 4, h * VS:(h + 1) * VS],
                        start=False, stop=True)
                    rc = rcpool.tile([P, 1], F32, tag="rc")
                    nc.vector.reciprocal(rc[:qn], pc[:qn, 64:65])
                    nc.vector.tensor_scalar_mul(
                        ctx_tm[:qn, qt, h * 64:(h + 1) * 64],
                        pc[:qn, 0:64], rc[:qn])

            def emit_ctxT_piece(ctx_tm, ctx_fm, kk):
                for ti, (t0, pt) in enumerate(TT):
                    pst = psA.tile([P, 512], BF16, tag="pA")
                    nc.tensor.transpose(
                        pst[:, :pt], ctx_tm[:pt, ti, kk * P:(kk + 1) * P],
                        identb[:pt, :pt])
                    nc.vector.tensor_copy(ctx_fm[:, kk, t0:t0 + pt],
                                          pst[:, :pt])

            def stage_outproj_ln2(b, ctx_fm, xb):
                # token-major out-projection + residual -> x2 (bf16), LN2 stats
                x2 = rpool.tile([P, 5, D], BF16, tag="resid", name=f"x2{b}")
                stats2 = ln_new_stats()
                for ci in range(2):
                    wb = wpool.tile([P, KK, 512], FP8, tag="wblk")
                    nc.sync.dma_start(wb[:], wo_r[:, :, ci * 512:(ci + 1) * 512])
                    for ti, (t0, pt) in enumerate(TT):
                        ps = psA.tile([P, 512], F32, tag="pA")
                        for kp in range(4):
                            nc.tensor.matmul(
                                ps[:pt], ctx_fm[:, 2 * kp:2 * kp + 2, t0:t0 + pt],
                                wb[:, 2 * kp:2 * kp + 2, :],
                                start=(kp == 0), stop=False, perf_mode=DR)
                        nc.tensor.matmul(
                            ps[:pt], ones8[:, :pt], t_bo[:, ci * 512:(ci + 1) * 512],
                            start=False, stop=True)
                        nc.vector.scalar_tensor_tensor(
                            x2[:pt, ti, ci * 512:(ci + 1) * 512], ps[:pt],
                            1.0 / (CTX_SCALE * WQ_SCALE),
                            xb[:pt, ti, ci * 512:(ci + 1) * 512],
                            OP.mult, OP.add)
                        if ci == 1:
                            ln_tile_stats(stats2, x2, ti, pt)
                return x2, stats2

            def emit_mlp1_chunk(b, m, xn2_fm, h1):
                blk, mi = m // 4, m % 4
                if mi == 0:
                    wb = m1pool.tile([P, KK, 512], BF16, tag="wm1",
                                     name=f"w1_{b}_{blk}")
                    nc.sync.dma_start(wb[:], w1_r[:, :, blk * 512:(blk + 1) * 512])
                    emit_mlp1_chunk.wb = wb
                wb = emit_mlp1_chunk.wb
                ps = psS.tile([P, 2, 512], F32, tag="pS")
                for qi, (q0, qn) in enumerate(QC):
                    for kk in range(KK):
                        nc.tensor.matmul(
                            ps[:, qi, :qn], wb[:, kk, mi * P:(mi + 1) * P],
                            xn2_fm[:, kk, q0:q0 + qn],
                            start=(kk == 0), stop=(kk == KK - 1))
                h14 = h1[:, :, 0:578].rearrange("p t (c q) -> p t c q", q=289)
                nc.scalar.activation(
                    h14[:, m], ps[:, :, :289], _GELU, bias=b1_sb[:, m:m + 1])

            def emit_mlp2_group(b, idx, h1, mlp_fm, x2):
                # one (m-chunk, token-half) accumulation group of h1 @ w2
                m, qi = idx // 2, idx % 2
                if qi == 0:
                    wb = m2pool.tile([P, FK, P], BF16, tag="wm2")
                    nc.sync.dma_start(wb[:], w2_r[:, :, m * P:(m + 1) * P])
                    emit_mlp2_group.wb = wb
                wb = emit_mlp2_group.wb
                q0, qn = QC[qi]
                ps = psA.tile([P, 512], F32, tag="pA")
                for kk2 in range(FK):
                    nc.tensor.matmul(
                        ps[:, :qn], wb[:, kk2], h1[:, kk2, q0:q0 + qn],
                        start=(kk2 == 0), stop=(kk2 == FK - 1))
                nc.scalar.activation(
                    mlp_fm[:, m, q0:q0 + qn], ps[:, :qn],
                    AF.Identity, bias=b2_sb[:, m:m + 1])

            def flush_mlp2_out(b, m, mlp_fm, x2):
                # transpose chunk m back to token-major, add residual, store
                for ti, (t0, pt) in enumerate(TT):
                    rp = min(pt, S - t0)
                    pst = psA.tile([P, 512], BF16, tag="pA")
                    nc.tensor.transpose(
                        pst[:pt, :P], mlp_fm[:, m, t0:t0 + pt], identb[:])
                    og = opool.tile([P, P], F32, tag="ostg")
                    nc.vector.tensor_tensor(
                        og[:pt], pst[:pt, :P],
                        x2[:pt, ti, m * P:(m + 1) * P], OP.add)
                    nc.sync.dma_start(
                        y_d[b, t0:t0 + rp, m * P:(m + 1) * P], og[:rp])

            # ---- batch prep (load + LN1 + QKV), splittable into units so it
            # can be spread under the previous batch's attention ----
            bstate = {}

            def prep_units(b):
                st = {}
                bstate[b] = st

                def u_load():
                    st["xb"] = stage_load(b)
                    st["stats"] = ln_new_stats()
                    for ti, (t0, pt) in enumerate(TT):
                        ln_tile_stats(st["stats"], st["xb"], ti, pt)

                def u_ln():
                    xn_fm = xnpool.tile([P, KK, SP], FP8, tag="xn_fm", bufs=1)
                    st["xn_fm"] = xn_fm
                    ln_finalize(st["stats"], 0, 5)
                    ln_apply_tiles(st["stats"], st["xb"], g1_sb, gb1_sb,
                                   xn_fm, (0, 1, 2, 3, 4))

                return [u_load, u_ln,
                        lambda: emit_qk_block(st, "q", 0),
                        lambda: emit_qk_block(st, "q", 1),
                        lambda: emit_qk_block(st, "k", 0),
                        lambda: emit_qk_block(st, "k", 1),
                        lambda: emit_v_block(st, 0),
                        lambda: emit_v_block(st, 1)]

            def xb_of(b):
                return bstate[b]["xb"]

            # ---- main schedule ----
            # slot b: [MLP1(b-1) gelu-run] ; [attention(b) || MLP2(b-1) ||
            # prep(b+1)] ; [ctxT (in-loop) / outproj / LN2(b) || MLP2 tail]
            prev = None   # (xn2_fm, x2) of batch b-1
            for u in prep_units(0):
                u()
            for slot in range(BL + 1):
                b = slot if slot < BL else None
                pb = slot - 1 if slot >= 1 else None

                # phase 1: MLP1(pb) — contiguous gelu run on Act
                if pb is not None:
                    pxn2, px2 = prev
                    h1 = mpool.tile([P, FK, SP], BF16, tag="h1")
                    for m in range(FK):
                        emit_mlp1_chunk(pb, m, pxn2, h1)
                    mlp_fm = mpool.tile([P, KK, SP], BF16, tag="mlp_fm")

                # MLP2(pb) pump: one (m, half) group per call, with the
                # token-major writeback of chunk m-1 skewed one group behind
                mq = list(range(2 * KK)) if pb is not None else []

                def pump_mlp2():
                    if not mq:
                        return
                    idx = mq.pop(0)
                    m, qi = idx // 2, idx % 2
                    if qi == 0 and m > 0:
                        flush_mlp2_out(pb, m - 1, mlp_fm, px2)
                    emit_mlp2_group(pb, idx, h1, mlp_fm, px2)

                def pump_rest():
                    while mq:
                        pump_mlp2()
                    if pb is not None:
                        flush_mlp2_out(pb, KK - 1, mlp_fm, px2)

                # phase 2: attention(b) || MLP2(pb) || prep(b+1)
                if b is not None:
                    st = bstate[b]
                    q_fm, k_fm, v_sb = st["q"], st["k"], st["v"]
                    ctx_tm = apool.tile([P, 5, H * 64], BF16, tag="ctxt",
                                        bufs=1)
                    ctx_fm = apool.tile([P, KK, SP], FP8, tag="ctxf", bufs=1)
                    units = prep_units(b + 1) if b + 1 < BL else []
                    for h in range(H):
                        es = emit_scores(h, q_fm, k_fm)
                        if h < 13:
                            pump_mlp2()
                        emit_pv(h, es, v_sb, ctx_tm)
                        if h % 2 == 1:
                            emit_ctxT_piece(ctx_tm, ctx_fm, h // 2)
                            if units:
                                units.pop(0)()
                    for u in units:
                        u()
                else:
                    pump_rest()

                # phase 3: out-projection + LN2, with remaining MLP2 pumped in
                if b is not None:
                    x2 = rpool.tile([P, 5, D], BF16, tag="resid",
                                    name=f"x2{b}")
                    stats2 = ln_new_stats()
                    for ci in range(2):
                        wb = wpool.tile([P, KK, 512], FP8, tag="wblk")
                        nc.sync.dma_start(wb[:],
                                          wo_r[:, :, ci * 512:(ci + 1) * 512])
                        for ti, (t0, pt) in enumerate(TT):
                            ps = psA.tile([P, 512], F32, tag="pA")
                            for kp in range(4):
                                nc.tensor.matmul(
                                    ps[:pt],
                                    ctx_fm[:, 2 * kp:2 * kp + 2, t0:t0 + pt],
                                    wb[:, 2 * kp:2 * kp + 2, :],
                                    start=(kp == 0), stop=False, perf_mode=DR)
                            nc.tensor.matmul(
                                ps[:pt], ones8[:, :pt],
                                t_bo[:, ci * 512:(ci + 1) * 512],
                                start=False, stop=True)
                            nc.vector.scalar_tensor_tensor(
                                x2[:pt, ti, ci * 512:(ci + 1) * 512], ps[:pt],
                                1.0 / (CTX_SCALE * WQ_SCALE),
                                xb_of(b)[:pt, ti, ci * 512:(ci + 1) * 512],
                                OP.mult, OP.add)
                            if ci == 1:
                                ln_tile_stats(stats2, x2, ti, pt)
                            if ti % 2 == 0:
                                pump_mlp2()
                    pump_rest()
                    xn2_fm = xn2pool.tile([P, KK, SP], BF16, tag="xn2_fm",
                                          bufs=1)
                    ln_finalize(stats2, 0, 5)
                    ln_apply_tiles(stats2, x2, g2_sb, gb2_sb, xn2_fm,
                                   (0, 1, 2, 3, 4))
                    prev = (xn2_fm, x2)

    nc.compile()
    return nc


def _get_nc():
    global _NC_CACHE
    if _NC_CACHE is None:
        _NC_CACHE = _build()
    return _NC_CACHE


def _pack_params_f32(f32):
    cols = [f32(n).reshape(-1, P).T for n in
            ("bq", "bk", "b2", "ln1_g", "ln1_b", "ln2_g", "ln2_b", "b1")]
    return np.ascontiguousarray(np.concatenate(cols, axis=1))


def _pack_params_fp8(f32):
    row = np.concatenate([
        (f32("bv") * WQ_SCALE).astype(E4NP),
        (f32("bo") * CTX_SCALE * WQ_SCALE).astype(E4NP),
        np.ones(P, np.float32).astype(E4NP),
    ])
    return np.ascontiguousarray(row[None, :])


def kernel(**inputs):
    nc = _get_nc()
    f32 = lambda n: np.ascontiguousarray(np.asarray(inputs[n], dtype=np.float32))

    x = f32("x")
    shared = {
        "wq8": np.ascontiguousarray((f32("wq") * WQ_SCALE).astype(E4NP)),
        "wk8": np.ascontiguousarray((f32("wk") * WQ_SCALE).astype(E4NP)),
        "wv8": np.ascontiguousarray((f32("wv") * WQ_SCALE).astype(E4NP)),
        "wo8": np.ascontiguousarray((f32("wo") * WQ_SCALE).astype(E4NP)),
        "w1b": np.ascontiguousarray(f32("w1").astype(BFNP)),
        "w2b": np.ascontiguousarray(f32("w2").astype(BFNP)),
        "params_f32": _pack_params_f32(f32),
        "params_fp8": _pack_params_fp8(f32),
    }
    in_maps = []
    for i in range(NCORES):
        m = dict(shared)
        m["x"] = np.ascontiguousarray(x[i * BL:(i + 1) * BL].astype(BFNP))
        in_maps.append(m)
    res = bass_utils.run_bass_kernel_spmd(nc, in_maps, core_ids=list(range(NCORES)))
    y = np.concatenate([res.results[i]["y"] for i in range(NCORES)], axis=0)
    return y.astype(np.float32)
